# revision 1
# baseline (speedup 1.0000x reference)
"""Quantized int8 conv2d (brevitas-style) on 8 TRN2 NeuronCores.

Data-parallel over batch (1 image / core). Per-tensor symmetric int8
quantization: local abs-max -> AllReduce(max) -> quantize -> 3x3 conv
(stride 1, pad 1) as PE col-tiled matmuls -> dequant + bias.

Key tricks:
- x is cached in SBUF as fp16 during the abs-max pass (single DRAM read).
- round(v) is computed as fp16(v + 1536) (fp16 RNE at the [1024,2048)
  binade has ulp=1 -> exact round-half-even, matching jnp.round). The
  +1536 offset rides through the conv matmuls and is cancelled exactly
  by a correction matmul per output row group using {0,1536} column
  patterns that replicate the zero-padding tap structure.
- Conv: output rows grouped by 4 (c = h%4). Each c is one PE column
  tile (0, 32c), all four concurrent, each accumulating into its own
  PSUM bank: 3 K=128 matmuls (kw taps; lhsT has zero row-blocks where
  the input row class is invalid) + 1 correction + (c=0/c=3) 3 windowed
  K=32 boundary matmuls against a staged copy of the adjacent block's
  edge row.
"""

import sys

if "/opt/trn_rl_repo" not in sys.path:
    sys.path.insert(0, "/opt/trn_rl_repo")

import numpy as np

import concourse.bass as bass
import concourse.bacc as bacc
import concourse.mybir as mybir
from concourse import tile
from concourse.bass_utils import run_bass_kernel_spmd

N_CORES = 8
C = 32
O = 32
H = 512
W = 512
F32 = mybir.dt.float32
F16 = mybir.dt.float16

MAXV = 127.0
RND = 1536.0

# per-kw output/rhs column windows: (out_start, rhs_start, n)
KW_COLS = {0: (1, 0, 511), 1: (0, 0, 512), 2: (0, 1, 511)}
G = 4  # q-blocks per DMA group


def build_nc(h=H):
    nc = bacc.Bacc(None, target_bir_lowering=False, debug=False)
    NQ = h // 4
    NG = NQ // G

    x_ext = nc.declare_dram_parameter("x", [C, h, W], F32, isOutput=False)
    w_ext = nc.declare_dram_parameter("weight", [O, C, 3, 3], F32, isOutput=False)
    b_ext = nc.declare_dram_parameter("bias", [O], F32, isOutput=False)
    out_ext = nc.declare_dram_parameter("out", [O, h, W], F32, isOutput=True)

    cc_in = nc.dram_tensor("cc_in", [128], F32)
    cc_out = nc.dram_tensor("cc_out", [128], F32, addr_space="Shared")

    with tile.TileContext(nc) as tc:
        with (
            tc.tile_pool(name="persist", bufs=1) as persist,
            tc.tile_pool(name="stage", bufs=3) as stage,
            tc.tile_pool(name="qx", bufs=6) as qxp,
            tc.tile_pool(name="outp", bufs=3) as outp,
            tc.tile_pool(name="spp", bufs=3) as spp,
            tc.tile_pool(name="snp", bufs=3) as snp,
            tc.tile_pool(name="ps0", bufs=2, space="PSUM") as psp0,
            tc.tile_pool(name="ps1", bufs=2, space="PSUM") as psp1,
            tc.tile_pool(name="ps2", bufs=2, space="PSUM") as psp2,
            tc.tile_pool(name="ps3", bufs=2, space="PSUM") as psp3,
        ):
            psps = [psp0, psp1, psp2, psp3]
            # ---------------- persistent SBUF tensors ----------------
            x4 = persist.tile([128, NQ * W], F16)
            maxes = persist.tile([128, NQ], F32)
            wsb = persist.tile([128, 288], F32)
            qw = persist.tile([128, 288], F16)
            tq = persist.tile([128, 288], F16)
            cw = persist.tile([128, 288], F16)
            w4 = persist.tile([128, 3 * 128], F16)   # main lhsT: kw blocks of (c,o)
            cwM = persist.tile([96, 3 * 128], F16)   # corr lhsT variants of (c,o)
            cb4 = persist.tile([96, W], F16)         # corr rhs patterns
            ones_l = persist.tile([1, 128], F32)
            bias_sb = persist.tile([128, 1], F32)
            gmax = persist.tile([128, 1], F32)
            gmax2 = persist.tile([128, 1], F32)
            gmaxr = persist.tile([1, 128], F32)
            wred = persist.tile([128, 1], F32)
            wredr = persist.tile([1, 128], F32)
            sg = persist.tile([1, 1], F32)
            sw = persist.tile([1, 1], F32)
            inv = persist.tile([1, 1], F32)
            invw = persist.tile([1, 1], F32)
            cwi = persist.tile([1, 1], F32)
            cqi = persist.tile([1, 1], F32)
            dqi = persist.tile([1, 1], F32)
            bc_in = persist.tile([1, 4], F32)
            bvec = persist.tile([128, 4], F32)
            cw_ap = persist.tile([128, 1], F32)
            s01 = persist.tile([128, 96], F16)
            s12 = persist.tile([128, 96], F16)
            sall = persist.tile([128, 96], F16)

            # ---------------- weight path (local) --------------------
            wv = w_ext[:, :, :, :].rearrange("o i kh kw -> i kh kw o")
            for g in range(4):
                nc.sync.dma_start(out=wsb[32 * g : 32 * g + 32, :], in_=wv)
            for cix in range(4):
                nc.sync.dma_start(
                    out=bias_sb[32 * cix : 32 * cix + 32, :], in_=b_ext[:, None]
                )
            nc.gpsimd.memset(ones_l[:, :], 1.0)
            nc.gpsimd.memset(w4[:, :], 0.0)
            nc.gpsimd.memset(cwM[:, :], 0.0)
            nc.gpsimd.memset(cb4[:, :], RND)
            nc.gpsimd.memset(cb4[0:32, 0:1], 0.0)
            nc.gpsimd.memset(cb4[64:96, W - 1 : W], 0.0)

            # sw = max |w| (X-reduce, fold partitions to a row, reduce again)
            nc.vector.tensor_reduce(
                out=wred[:, :], in_=wsb[:, :], axis=mybir.AxisListType.X,
                op=mybir.AluOpType.max, apply_absolute_value=True,
            )
            nc.sync.dma_start(out=wredr[0:1, 0:128], in_=wred[:, 0:1])
            nc.vector.tensor_reduce(
                out=sw[:, :], in_=wredr[:, :], axis=mybir.AxisListType.X,
                op=mybir.AluOpType.max,
            )
            nc.vector.reciprocal(invw[:, :], sw[:, :])
            nc.vector.tensor_scalar_mul(cwi[:, :], invw[:, :], MAXV)

            if True:
                bps = psp0.tile([128, 4], F32, tag="pst0")
                nc.tensor.matmul(bps[:, 0:1], ones_l[:, :], cwi[:, :])
                nc.vector.tensor_copy(cw_ap[:, :], bps[:, 0:1])

                # qw = round(w * 127/sw) via fp16 +1536 trick
                nc.scalar.activation(
                    out=tq[:, :], in_=wsb[:, :],
                    func=mybir.ActivationFunctionType.Copy,
                    scale=cw_ap[:, 0:1], bias=RND,
                )
                with nc.allow_low_precision("int8 values exact in fp16"):
                    nc.vector.tensor_scalar_add(qw[:, :], tq[:, :], -RND)
                    nc.vector.tensor_add(s01[:, :], qw[:, 0:96], qw[:, 96:192])
                    nc.vector.tensor_add(s12[:, :], qw[:, 96:192], qw[:, 192:288])
                    nc.vector.tensor_add(sall[:, :], s01[:, :], qw[:, 192:288])
                    nc.vector.tensor_scalar_mul(cw[:, 0:96], sall[:, :], -1.0)
                    nc.vector.tensor_scalar_mul(cw[:, 96:192], s12[:, :], -1.0)
                    nc.vector.tensor_scalar_mul(cw[:, 192:288], s01[:, :], -1.0)
                    # main lhsT: w4[32*hm+i, kw*128+c*32+o] = qw[o,i,hm-c+1,kw]
                    for cix in range(4):
                        for kw in range(3):
                            for kh in range(3):
                                hm = cix + kh - 1
                                if not (0 <= hm <= 3):
                                    continue
                                nc.vector.tensor_copy(
                                    w4[32 * hm : 32 * hm + 32,
                                       kw * 128 + cix * 32 : kw * 128 + cix * 32 + 32],
                                    qw[0:32, kh * 96 + kw * 32 : kh * 96 + kw * 32 + 32],
                                )
                    # corr lhsT: cwM[32*kw+i, vv*128+c*32+o], vv=0 mid,1 q0,2 qlast
                    for vv in range(3):
                        for cix in range(4):
                            v = 1 if (vv == 1 and cix == 0) else (
                                2 if (vv == 2 and cix == 3) else 0)
                            for kw in range(3):
                                nc.vector.tensor_copy(
                                    cwM[32 * kw : 32 * kw + 32,
                                        vv * 128 + cix * 32 : vv * 128 + cix * 32 + 32],
                                    cw[0:32, v * 96 + kw * 32 : v * 96 + kw * 32 + 32],
                                )

                # ------------- pass 1: stream x, absmax + fp16 cache --
                for q in range(NQ):
                    stg = stage.tile([128, W], F32)
                    xv = x_ext[:, 4 * q : 4 * q + 4, :].rearrange("i hm w -> hm i w")
                    eng = (nc.sync, nc.scalar, nc.gpsimd)[q % 3]
                    eng.dma_start(out=stg[:, :], in_=xv)
                    nc.scalar.activation(
                        out=x4[:, q * W : (q + 1) * W], in_=stg[:, :],
                        func=mybir.ActivationFunctionType.Copy,
                    )
                    nc.vector.tensor_reduce(
                        out=maxes[:, q : q + 1], in_=stg[:, :],
                        axis=mybir.AxisListType.X,
                        op=mybir.AluOpType.max, apply_absolute_value=True,
                    )

                nc.vector.tensor_reduce(
                    out=gmax[:, :], in_=maxes[:, :], axis=mybir.AxisListType.X,
                    op=mybir.AluOpType.max,
                )

                # ------------- all-reduce(max) across 8 cores ---------
                nc.sync.dma_start(out=cc_in[:, None], in_=gmax[:, :])
                nc.gpsimd.collective_compute(
                    "AllReduce", mybir.AluOpType.max,
                    replica_groups=[list(range(N_CORES))],
                    ins=[cc_in[:].opt()], outs=[cc_out[:].opt()],
                )
                nc.sync.dma_start(out=gmax2[:, :], in_=cc_out[:, None])
                nc.sync.dma_start(out=gmaxr[0:1, 0:128], in_=gmax2[:, 0:1])
                nc.vector.tensor_reduce(
                    out=sg[:, :], in_=gmaxr[:, :], axis=mybir.AxisListType.X,
                    op=mybir.AluOpType.max,
                )

                nc.vector.reciprocal(inv[:, :], sg[:, :])
                nc.vector.tensor_scalar_mul(cqi[:, :], inv[:, :], MAXV)
                nc.vector.tensor_mul(dqi[:, :], sg[:, :], sw[:, :])
                nc.vector.tensor_scalar_mul(dqi[:, :], dqi[:, :], 1.0 / (MAXV * MAXV))
                nc.vector.tensor_copy(bc_in[:, 0:1], cqi[:, :])
                nc.vector.tensor_copy(bc_in[:, 1:2], dqi[:, :])
                bps2 = psp1.tile([128, 4], F32, tag="pst1")
                nc.tensor.matmul(bps2[:, 0:2], ones_l[:, :], bc_in[:, 0:2])
                nc.vector.tensor_copy(bvec[:, 0:2], bps2[:, 0:2])
            cq_ap = bvec[:, 0:1]
            dq_ap = bvec[:, 1:2]

            # ---------------- pass 2 ----------------------------------
            qx_tiles = {}

            def quantize_block(j):
                t = qxp.tile([128, W], F16)
                nc.scalar.activation(
                    out=t[:, :], in_=x4[:, j * W : (j + 1) * W],
                    func=mybir.ActivationFunctionType.Copy,
                    scale=cq_ap, bias=RND,
                )
                qx_tiles[j] = t

            quantize_block(0)
            quantize_block(1)

            ot4 = None
            for q in range(NQ):
                if q + 2 <= NQ - 1:
                    quantize_block(q + 2)

                sp32 = sn32 = None
                if q > 0:
                    sp32 = spp.tile([32, W], F16)
                    nc.gpsimd.dma_start(out=sp32[:, :], in_=qx_tiles[q - 1][96:128, :])
                if q < NQ - 1:
                    sn32 = snp.tile([32, W], F16)
                    nc.gpsimd.dma_start(out=sn32[:, :], in_=qx_tiles[q + 1][0:32, :])

                cur = qx_tiles[q]
                pst = psps[q % 4].tile([128, W], F32, tag=f"pst{q % 4}")
                vv = 1 if q == 0 else (2 if q == NQ - 1 else 0)
                mms = []
                for kw in (1, 0, 2):
                    oc0, rc0, nn = KW_COLS[kw]
                    mms.append(
                        (w4[0:128, kw * 128 : kw * 128 + 128],
                         cur[0:128, rc0 : rc0 + nn],
                         pst[0:128, oc0 : oc0 + nn], (0, 0))
                    )
                if sp32 is not None:
                    for kw in (1, 0, 2):
                        oc0, rc0, nn = KW_COLS[kw]
                        mms.append(
                            (qw[0:32, kw * 32 : kw * 32 + 32],  # kh=0 -> c=0
                             sp32[0:32, rc0 : rc0 + nn],
                             pst[0:32, oc0 : oc0 + nn], (0, 0))
                        )
                if sn32 is not None:
                    for kw in (1, 0, 2):
                        oc0, rc0, nn = KW_COLS[kw]
                        mms.append(
                            (qw[0:32, 192 + kw * 32 : 192 + kw * 32 + 32],  # kh=2 -> c=3
                             sn32[0:32, rc0 : rc0 + nn],
                             pst[96:128, oc0 : oc0 + nn], (0, 96))
                        )
                mms.append(
                    (cwM[0:96, vv * 128 : vv * 128 + 128], cb4[0:96, 0:W],
                     pst[0:128, 0:W], (0, 0))
                )
                for mi, (lhsT, rhs, outap, tpos) in enumerate(mms):
                    nc.tensor.matmul(
                        outap, lhsT, rhs,
                        start=(mi == 0), stop=(mi == len(mms) - 1),
                        tile_position=tpos,
                    )

                ot4 = outp.tile([128, W], F32)
                nc.vector.tensor_scalar(
                    out=ot4[:, :], in0=pst[:, :],
                    scalar1=dq_ap, scalar2=bias_sb[:, 0:1],
                    op0=mybir.AluOpType.mult, op1=mybir.AluOpType.add,
                )
                ov = out_ext[:, 4 * q : 4 * q + 4, :].rearrange("o hm w -> hm o w")
                nc.sync.dma_start(out=ov, in_=ot4[:, :])

    nc.finalize()
    return nc


_NC_CACHE = {}


def kernel(x, weight, bias):
    x = np.ascontiguousarray(x, dtype=np.float32)
    weight = np.ascontiguousarray(weight, dtype=np.float32)
    bias = np.ascontiguousarray(bias, dtype=np.float32)
    if "nc" not in _NC_CACHE:
        _NC_CACHE["nc"] = build_nc()
    nc = _NC_CACHE["nc"]
    in_maps = [
        {"x": x[i], "weight": weight, "bias": bias} for i in range(N_CORES)
    ]
    res = run_bass_kernel_spmd(nc, in_maps, core_ids=list(range(N_CORES)))
    outs = [res.results[i]["out"] for i in range(N_CORES)]
    return np.stack(outs, axis=0)


if __name__ == "__main__":
    build_nc(h=32)
    print("build ok")



# revision 6
# speedup vs baseline: 1.2961x; 1.2961x over previous
"""Quantized int8 conv2d (brevitas-style) on 8 TRN2 NeuronCores.

Data-parallel over batch (1 image / core). Per-tensor symmetric int8
quantization: local abs-max -> AllReduce(max) -> quantize -> 3x3 conv
(stride 1, pad 1) as PE col-tiled matmuls -> dequant + bias.

v2 design:
- Pass 1: 8x 4MB SWDGE DMAs (gpsimd) with inline fp32->fp16 cast land x
  directly in the SBUF cache (engines 4-15, ~320 GB/s, no scalar pass).
  Vector absmax per 4MB group rides along.
- round(v) = fp16(v + 1536) - 1536 (fp16 RNE at the [1024,2048) binade
  has ulp=1 -> exact round-half-even). The offset is removed with a
  cheap vector subtract so quantized values are exact int8-valued fp16
  and zero-padding at image borders needs no correction matmuls.
- Conv: output rows grouped by 4 (c = h%4), partition = 32*(h%4)+ch.
  Per block: 3 full K=128 matmuls (one per kw; h-taps folded into the
  (hm -> c) block structure of the lhsT) + up to 6 K=32 boundary
  matmuls that read the adjacent block's edge row DIRECTLY from its
  SBUF tile (partitions 96-127 / 0-31) using PE row/col tile placement
  (no staging DMA; sp/sn pairs run on disjoint 32x32 subarrays).
- fp16 output (halves store traffic; well within 2e-2), upcast on host.
  Stores rotate gpsimd/sync/scalar queues to use all 16 DMA engines.
"""

import sys

if "/opt/trn_rl_repo" not in sys.path:
    sys.path.insert(0, "/opt/trn_rl_repo")

import numpy as np

import concourse.bass as bass
import concourse.bacc as bacc
import concourse.mybir as mybir
from concourse import tile
from concourse.bass_utils import run_bass_kernel_spmd

N_CORES = 8
C = 32
O = 32
H = 512
W = 512
F32 = mybir.dt.float32
F16 = mybir.dt.float16

MAXV = 127.0
RND = 1536.0


def build_nc(h=H):
    nc = bacc.Bacc(None, target_bir_lowering=False, debug=False)
    NQ = h // 4          # 4-row blocks
    LG = min(16, NQ)     # blocks per load group (16 -> 4MB casting DMAs)
    NLG = NQ // LG
    QG = min(4, NQ)      # blocks per quantize group
    NQG = NQ // QG

    x_ext = nc.declare_dram_parameter("x", [C, h, W], F32, isOutput=False)
    w_ext = nc.declare_dram_parameter("weight", [O, C, 3, 3], F32, isOutput=False)
    b_ext = nc.declare_dram_parameter("bias", [O], F32, isOutput=False)
    out_ext = nc.declare_dram_parameter("out", [O, h, W], F16, isOutput=True)

    cc_in = nc.dram_tensor("cc_in", [128], F32)
    cc_out = nc.dram_tensor("cc_out", [128], F32, addr_space="Shared")

    with tile.TileContext(nc) as tc:
        with (
            tc.tile_pool(name="persist", bufs=1) as persist,
            tc.tile_pool(name="tp", bufs=2) as tp,
            tc.tile_pool(name="qx", bufs=4) as qxp,
            tc.tile_pool(name="outp", bufs=3) as outp,
            tc.tile_pool(name="psp", bufs=4, space="PSUM") as psp,
            tc.tile_pool(name="psx", bufs=2, space="PSUM") as psx,
        ):
            # ---------------- persistent SBUF tensors ----------------
            x4 = persist.tile([128, NQ * W], F16)
            maxes = persist.tile([128, NLG], F32)
            wsb = persist.tile([128, 288], F32)
            tq = persist.tile([128, 288], F16)
            qw = persist.tile([128, 288], F16)
            w4 = persist.tile([128, 3 * 128], F16)  # main lhsT: kw blocks (c,o)
            bw = persist.tile([128, 96], F16)       # boundary lhsT
            ones_l = persist.tile([1, 128], F32)
            bias_sb = persist.tile([128, 1], F32)
            gmax = persist.tile([128, 1], F32)
            gmaxr = persist.tile([1, 128], F32)
            wred = persist.tile([128, 1], F32)
            wredr = persist.tile([1, 128], F32)
            sg = persist.tile([1, 1], F32)
            sw_ = persist.tile([1, 1], F32)
            inv = persist.tile([1, 1], F32)
            invw = persist.tile([1, 1], F32)
            cwi = persist.tile([1, 1], F32)
            cqi = persist.tile([1, 1], F32)
            dqi = persist.tile([1, 1], F32)
            bc_in = persist.tile([1, 2], F32)
            bvec = persist.tile([128, 2], F32)
            cw_ap = persist.tile([128, 1], F32)

            # ---------------- weight path (local) --------------------
            wv = w_ext[:, :, :, :].rearrange("o i kh kw -> i kh kw o")
            for g in range(4):
                eng = (nc.sync, nc.scalar)[g % 2]
                eng.dma_start(out=wsb[32 * g : 32 * g + 32, :], in_=wv)
            for g in range(4):
                eng = (nc.scalar, nc.sync)[g % 2]
                eng.dma_start(
                    out=bias_sb[32 * g : 32 * g + 32, :], in_=b_ext[:, None]
                )
            nc.gpsimd.memset(ones_l[:, :], 1.0)
            nc.gpsimd.memset(w4[:, :], 0.0)
            nc.gpsimd.memset(bw[:, :], 0.0)

            # sw = max |w| (X-reduce, fold partitions to a row, reduce again)
            nc.vector.tensor_reduce(
                out=wred[:, :], in_=wsb[:, :], axis=mybir.AxisListType.X,
                op=mybir.AluOpType.max, apply_absolute_value=True,
            )
            nc.sync.dma_start(out=wredr[0:1, 0:128], in_=wred[:, 0:1])
            nc.vector.tensor_reduce(
                out=sw_[:, :], in_=wredr[:, :], axis=mybir.AxisListType.X,
                op=mybir.AluOpType.max,
            )
            nc.vector.reciprocal(invw[:, :], sw_[:, :])
            nc.vector.tensor_scalar_mul(cwi[:, :], invw[:, :], MAXV)

            bps = psx.tile([128, 2], F32, tag="bcast")
            nc.tensor.matmul(bps[:, 0:1], ones_l[:, :], cwi[:, :])
            nc.vector.tensor_copy(cw_ap[:, :], bps[:, 0:1])

            # qw = round(w * 127/sw) via fp16 +1536 trick, then remove offset
            nc.scalar.activation(
                out=tq[:, :], in_=wsb[:, :],
                func=mybir.ActivationFunctionType.Copy,
                scale=cw_ap[:, 0:1], bias=RND,
            )
            with nc.allow_low_precision("int8 values exact in fp16"):
                nc.vector.tensor_scalar_add(qw[:, :], tq[:, :], -RND)
                # main lhsT: w4[32*hm+i, kw*128+c*32+o] = qw[o,i,hm-c+1,kw]
                for cix in range(4):
                    for kw in range(3):
                        for kh in range(3):
                            hm = cix + kh - 1
                            if not (0 <= hm <= 3):
                                continue
                            nc.vector.tensor_copy(
                                w4[32 * hm : 32 * hm + 32,
                                   kw * 128 + cix * 32 : kw * 128 + cix * 32 + 32],
                                qw[0:32, kh * 96 + kw * 32 : kh * 96 + kw * 32 + 32],
                            )
                # boundary lhsT:
                #   bw[96+i, kw*32+o] = qw[o,i,kh=0,kw]  (sp: prev block row 3)
                #   bw[i,    kw*32+o] = qw[o,i,kh=2,kw]  (sn: next block row 0)
                for kw in range(3):
                    nc.vector.tensor_copy(
                        bw[96:128, kw * 32 : kw * 32 + 32],
                        qw[96:128, 0 * 96 + kw * 32 : 0 * 96 + kw * 32 + 32],
                    )
                    nc.vector.tensor_copy(
                        bw[0:32, kw * 32 : kw * 32 + 32],
                        qw[0:32, 2 * 96 + kw * 32 : 2 * 96 + kw * 32 + 32],
                    )

            # ------------- pass 1: cast-DMA x into SBUF + absmax ------
            # 4 per-hm DMAs per group: 3-dim APs, 32-partition targets that
            # together cover all 16 SDMA engines; fp32->fp16 cast inline.
            for g in range(NLG):
                for hm in range(4):
                    xv = x_ext[
                        :, 4 * LG * g + hm : 4 * LG * (g + 1) : 4, :
                    ]
                    nc.gpsimd.dma_start(
                        out=x4[32 * hm : 32 * hm + 32,
                               g * LG * W : (g + 1) * LG * W].rearrange(
                            "p (j w) -> p j w", j=LG
                        ),
                        in_=xv.rearrange("i j w -> i j w"),
                    )
                nc.vector.tensor_reduce(
                    out=maxes[:, g : g + 1],
                    in_=x4[:, g * LG * W : (g + 1) * LG * W],
                    axis=mybir.AxisListType.X,
                    op=mybir.AluOpType.max, apply_absolute_value=True,
                )
            nc.vector.tensor_reduce(
                out=gmax[:, :], in_=maxes[:, :], axis=mybir.AxisListType.X,
                op=mybir.AluOpType.max,
            )

            # ------------- all-reduce(max) across 8 cores -------------
            nc.sync.dma_start(out=cc_in[:, None], in_=gmax[:, :])
            nc.gpsimd.collective_compute(
                "AllReduce", mybir.AluOpType.max,
                replica_groups=[list(range(N_CORES))],
                ins=[cc_in[:].opt()], outs=[cc_out[:].opt()],
            )
            nc.sync.dma_start(out=gmaxr[0:1, 0:128], in_=cc_out[None, :])
            nc.vector.tensor_reduce(
                out=sg[:, :], in_=gmaxr[:, :], axis=mybir.AxisListType.X,
                op=mybir.AluOpType.max,
            )
            nc.vector.reciprocal(inv[:, :], sg[:, :])
            nc.vector.tensor_scalar_mul(cqi[:, :], inv[:, :], MAXV)
            nc.vector.tensor_mul(dqi[:, :], sg[:, :], sw_[:, :])
            nc.vector.tensor_scalar_mul(dqi[:, :], dqi[:, :], 1.0 / (MAXV * MAXV))
            nc.vector.tensor_copy(bc_in[:, 0:1], cqi[:, :])
            nc.vector.tensor_copy(bc_in[:, 1:2], dqi[:, :])
            bps2 = psx.tile([128, 2], F32, tag="bcast")
            nc.tensor.matmul(bps2[:, 0:2], ones_l[:, :], bc_in[:, 0:2])
            nc.vector.tensor_copy(bvec[:, 0:2], bps2[:, 0:2])
            cq_ap = bvec[:, 0:1]
            dq_ap = bvec[:, 1:2]

            # ---------------- pass 2 ----------------------------------
            qx_tiles = {}

            def quantize_group(gq):
                t = tp.tile([128, QG * W], F16)
                nc.scalar.activation(
                    out=t[:, :], in_=x4[:, gq * QG * W : (gq + 1) * QG * W],
                    func=mybir.ActivationFunctionType.Copy,
                    scale=cq_ap, bias=RND,
                )
                qt = qxp.tile([128, QG * W], F16)
                with nc.allow_low_precision("int8 values exact in fp16"):
                    nc.vector.tensor_scalar_add(qt[:, :], t[:, :], -RND)
                qx_tiles[gq] = qt

            quantize_group(0)
            if NQG > 1:
                quantize_group(1)

            ot = None
            for q in range(NQ):
                gq, off = divmod(q, QG)
                off *= W
                if q % QG == 0 and gq + 2 <= NQG - 1:
                    quantize_group(gq + 2)

                cur = qx_tiles[gq]
                if q == 0:
                    prev_t = None
                elif q % QG:
                    prev_t, prev_off = cur, off - W
                else:
                    prev_t, prev_off = qx_tiles[gq - 1], (QG - 1) * W
                if q == NQ - 1:
                    next_t = None
                elif (q % QG) != QG - 1:
                    next_t, next_off = cur, off + W
                else:
                    next_t, next_off = qx_tiles[gq + 1], 0

                pst = psp.tile([128, W], F32, tag="pst")
                # (lhsT, rhs, out); kw order 1,0,2; boundary sp/sn interleaved
                mms = [
                    (w4[:, 128:256], cur[:, off : off + 512], pst[:, 0:512],
                     (0, 0)),
                    (w4[:, 0:128], cur[:, off : off + 511], pst[:, 1:512],
                     (0, 0)),
                    (w4[:, 256:384], cur[:, off + 1 : off + 512],
                     pst[:, 0:511], (0, 0)),
                ]
                bnd = []
                if prev_t is not None:
                    p0 = prev_off
                    bnd.append([
                        (bw[96:128, 32:64], prev_t[96:128, p0 : p0 + 512],
                         pst[0:32, 0:512], (96, 0)),
                        (bw[96:128, 0:32], prev_t[96:128, p0 : p0 + 511],
                         pst[0:32, 1:512], (96, 0)),
                        (bw[96:128, 64:96], prev_t[96:128, p0 + 1 : p0 + 512],
                         pst[0:32, 0:511], (96, 0)),
                    ])
                if next_t is not None:
                    n0 = next_off
                    bnd.append([
                        (bw[0:32, 32:64], next_t[0:32, n0 : n0 + 512],
                         pst[96:128, 0:512], (0, 96)),
                        (bw[0:32, 0:32], next_t[0:32, n0 : n0 + 511],
                         pst[96:128, 1:512], (0, 96)),
                        (bw[0:32, 64:96], next_t[0:32, n0 + 1 : n0 + 512],
                         pst[96:128, 0:511], (0, 96)),
                    ])
                # interleave sp/sn per-kw so the (row 96, col 0) and
                # (row 0, col 3) subarray pairs run concurrently
                for trip in zip(*bnd):
                    mms.extend(trip)
                for mi, (lhsT, rhs, outap, tpos) in enumerate(mms):
                    nc.tensor.matmul(
                        outap, lhsT, rhs,
                        start=(mi == 0), stop=(mi == len(mms) - 1),
                        tile_position=tpos,
                    )

                ot = outp.tile([128, W], F16)
                nc.vector.tensor_scalar(
                    out=ot[:, :], in0=pst[:, :],
                    scalar1=dq_ap, scalar2=bias_sb[:, 0:1],
                    op0=mybir.AluOpType.mult, op1=mybir.AluOpType.add,
                )
                ov = out_ext[:, 4 * q : 4 * q + 4, :].rearrange(
                    "o hm w -> hm o w"
                )
                eng = (nc.gpsimd, nc.sync, nc.gpsimd, nc.scalar)[q % 4]
                eng.dma_start(out=ov, in_=ot[:, :])

    nc.finalize()
    return nc


_NC_CACHE = {}


def kernel(x, weight, bias):
    x = np.ascontiguousarray(x, dtype=np.float32)
    weight = np.ascontiguousarray(weight, dtype=np.float32)
    bias = np.ascontiguousarray(bias, dtype=np.float32)
    if "nc" not in _NC_CACHE:
        _NC_CACHE["nc"] = build_nc()
    nc = _NC_CACHE["nc"]
    in_maps = [
        {"x": x[i], "weight": weight, "bias": bias} for i in range(N_CORES)
    ]
    res = run_bass_kernel_spmd(nc, in_maps, core_ids=list(range(N_CORES)))
    outs = [
        np.asarray(res.results[i]["out"], dtype=np.float32)
        for i in range(N_CORES)
    ]
    return np.stack(outs, axis=0)


if __name__ == "__main__":
    build_nc(h=32)
    print("build ok")


# revision 7
# speedup vs baseline: 1.5700x; 1.2114x over previous
"""Quantized int8 conv2d (brevitas-style) on 8 TRN2 NeuronCores.

Data-parallel over batch (1 image / core). Per-tensor symmetric int8
quantization: local abs-max -> AllReduce(max) -> quantize -> 3x3 conv
(stride 1, pad 1) as PE col-tiled matmuls -> dequant + bias.

v2 design:
- Pass 1: 8x 4MB SWDGE DMAs (gpsimd) with inline fp32->fp16 cast land x
  directly in the SBUF cache (engines 4-15, ~320 GB/s, no scalar pass).
  Vector absmax per 4MB group rides along.
- round(v) = fp16(v + 1536) - 1536 (fp16 RNE at the [1024,2048) binade
  has ulp=1 -> exact round-half-even). The offset is removed with a
  cheap vector subtract so quantized values are exact int8-valued fp16
  and zero-padding at image borders needs no correction matmuls.
- Conv: output rows grouped by 4 (c = h%4), partition = 32*(h%4)+ch.
  Per block: 3 full K=128 matmuls (one per kw; h-taps folded into the
  (hm -> c) block structure of the lhsT) + up to 6 K=32 boundary
  matmuls that read the adjacent block's edge row DIRECTLY from its
  SBUF tile (partitions 96-127 / 0-31) using PE row/col tile placement
  (no staging DMA; sp/sn pairs run on disjoint 32x32 subarrays).
- fp16 output (halves store traffic; well within 2e-2), upcast on host.
  Stores rotate gpsimd/sync/scalar queues to use all 16 DMA engines.
"""

import sys

if "/opt/trn_rl_repo" not in sys.path:
    sys.path.insert(0, "/opt/trn_rl_repo")

import numpy as np

import concourse.bass as bass
import concourse.bacc as bacc
import concourse.mybir as mybir
from concourse import tile
from concourse.bass_utils import run_bass_kernel_spmd

N_CORES = 8
C = 32
O = 32
H = 512
W = 512
F32 = mybir.dt.float32
F16 = mybir.dt.float16
BF16 = mybir.dt.bfloat16

MAXV = 127.0
RND = 1536.0


def build_nc(h=H):
    nc = bacc.Bacc(None, target_bir_lowering=False, debug=False)
    NQ = h // 4          # 4-row blocks
    LG = min(16, NQ)     # blocks per load group (16 -> 4MB casting DMAs)
    NLG = NQ // LG
    QG = min(4, NQ)      # blocks per quantize group
    NQG = NQ // QG

    x_ext = nc.declare_dram_parameter("x", [C, h, W], F32, isOutput=False)
    # wt = weight.transpose(1,2,3,0) prepared on host: [i, kh, kw, o]
    w_ext = nc.declare_dram_parameter("wt", [C, 3 * 3 * O], F32, isOutput=False)
    b_ext = nc.declare_dram_parameter("bias4", [128], F32, isOutput=False)
    out_ext = nc.declare_dram_parameter("out", [O, h, W], F16, isOutput=True)

    cc_in = nc.dram_tensor("cc_in", [128], F32)
    cc_out = nc.dram_tensor("cc_out", [128], F32, addr_space="Shared")

    with tile.TileContext(nc) as tc:
        with (
            tc.tile_pool(name="persist", bufs=1) as persist,
            tc.tile_pool(name="tp", bufs=2) as tp,
            tc.tile_pool(name="qx", bufs=4) as qxp,
            tc.tile_pool(name="outp", bufs=3) as outp,
            tc.tile_pool(name="psp", bufs=4, space="PSUM") as psp,
            tc.tile_pool(name="psx", bufs=2, space="PSUM") as psx,
        ):
            # ---------------- persistent SBUF tensors ----------------
            x4 = persist.tile([128, NQ * W], F16)
            maxes = persist.tile([128, NQG], F32)
            wsb = persist.tile([128, 288], F32)
            tq = persist.tile([128, 288], F16)
            qw = persist.tile([128, 288], BF16)
            w4 = persist.tile([128, 3 * 128], BF16)  # main lhsT: kw blocks (c,o)
            bw = persist.tile([128, 96], BF16)       # boundary lhsT
            ones_l = persist.tile([1, 128], F32)
            bias_sb = persist.tile([128, 1], F32)
            gmax = persist.tile([128, 1], F32)
            gmaxr = persist.tile([1, 128], F32)
            wred = persist.tile([128, 1], F32)
            wredr = persist.tile([1, 128], F32)
            sg = persist.tile([1, 1], F32)
            sw_ = persist.tile([1, 1], F32)
            inv = persist.tile([1, 1], F32)
            invw = persist.tile([1, 1], F32)
            cwi = persist.tile([1, 1], F32)
            cqi = persist.tile([1, 1], F32)
            dqi = persist.tile([1, 1], F32)
            bc_in = persist.tile([1, 2], F32)
            bvec = persist.tile([128, 2], F32)
            cw_ap = persist.tile([128, 1], F32)

            # ---------------- weight path (local) --------------------
            for g in range(4):
                eng = (nc.sync, nc.scalar)[g % 2]
                eng.dma_start(out=wsb[32 * g : 32 * g + 32, :], in_=w_ext[:, :])
            nc.sync.dma_start(out=bias_sb[:, :], in_=b_ext[:, None])
            nc.gpsimd.memset(ones_l[:, :], 1.0)
            nc.gpsimd.memset(w4[:, :], 0.0)
            nc.gpsimd.memset(bw[:, :], 0.0)

            # sw = max |w| (X-reduce, fold partitions to a row, reduce again)
            nc.vector.tensor_reduce(
                out=wred[:, :], in_=wsb[:, :], axis=mybir.AxisListType.X,
                op=mybir.AluOpType.max, apply_absolute_value=True,
            )
            nc.sync.dma_start(out=wredr[0:1, 0:128], in_=wred[:, 0:1])
            nc.vector.tensor_reduce(
                out=sw_[:, :], in_=wredr[:, :], axis=mybir.AxisListType.X,
                op=mybir.AluOpType.max,
            )
            nc.vector.reciprocal(invw[:, :], sw_[:, :])
            nc.vector.tensor_scalar_mul(cwi[:, :], invw[:, :], MAXV)

            bps = psx.tile([128, 2], F32, tag="bcast")
            nc.tensor.matmul(bps[:, 0:1], ones_l[:, :], cwi[:, :])
            nc.vector.tensor_copy(cw_ap[:, :], bps[:, 0:1])

            # qw = round(w * 127/sw) via fp16 +1536 trick, then remove offset
            nc.scalar.activation(
                out=tq[:, :], in_=wsb[:, :],
                func=mybir.ActivationFunctionType.Copy,
                scale=cw_ap[:, 0:1], bias=RND,
            )
            with nc.allow_low_precision("int8 values exact in fp16"):
                nc.vector.tensor_scalar_add(qw[:, :], tq[:, :], -RND)
                # main lhsT: w4[32*hm+i, kw*128+c*32+o] = qw[o,i,hm-c+1,kw]
                for cix in range(4):
                    for kw in range(3):
                        for kh in range(3):
                            hm = cix + kh - 1
                            if not (0 <= hm <= 3):
                                continue
                            nc.vector.tensor_copy(
                                w4[32 * hm : 32 * hm + 32,
                                   kw * 128 + cix * 32 : kw * 128 + cix * 32 + 32],
                                qw[0:32, kh * 96 + kw * 32 : kh * 96 + kw * 32 + 32],
                            )
                # boundary lhsT:
                #   bw[96+i, kw*32+o] = qw[o,i,kh=0,kw]  (sp: prev block row 3)
                #   bw[i,    kw*32+o] = qw[o,i,kh=2,kw]  (sn: next block row 0)
                for kw in range(3):
                    nc.vector.tensor_copy(
                        bw[96:128, kw * 32 : kw * 32 + 32],
                        qw[96:128, 0 * 96 + kw * 32 : 0 * 96 + kw * 32 + 32],
                    )
                    nc.vector.tensor_copy(
                        bw[0:32, kw * 32 : kw * 32 + 32],
                        qw[0:32, 2 * 96 + kw * 32 : 2 * 96 + kw * 32 + 32],
                    )

            # ------------- pass 1: cast-DMA x into SBUF + absmax ------
            # 4 per-hm DMAs per group: 3-dim APs, 32-partition targets that
            # together cover all 16 SDMA engines; fp32->fp16 cast inline.
            for g in range(NLG):
                for hm in range(4):
                    xv = x_ext[
                        :, 4 * LG * g + hm : 4 * LG * (g + 1) : 4, :
                    ]
                    nc.gpsimd.dma_start(
                        out=x4[32 * hm : 32 * hm + 32,
                               g * LG * W : (g + 1) * LG * W].rearrange(
                            "p (j w) -> p j w", j=LG
                        ),
                        in_=xv.rearrange("i j w -> i j w"),
                    )
                for c0 in range(g * LG * W, (g + 1) * LG * W, QG * W):
                    gq = c0 // (QG * W)
                    nc.vector.tensor_reduce(
                        out=maxes[:, gq : gq + 1],
                        in_=x4[:, c0 : c0 + QG * W],
                        axis=mybir.AxisListType.X,
                        op=mybir.AluOpType.max, apply_absolute_value=True,
                    )
            nc.vector.tensor_reduce(
                out=gmax[:, :], in_=maxes[:, :], axis=mybir.AxisListType.X,
                op=mybir.AluOpType.max,
            )

            # ------------- all-reduce(max) across 8 cores -------------
            nc.sync.dma_start(out=cc_in[:, None], in_=gmax[:, :])
            nc.gpsimd.collective_compute(
                "AllReduce", mybir.AluOpType.max,
                replica_groups=[list(range(N_CORES))],
                ins=[cc_in[:].opt()], outs=[cc_out[:].opt()],
            )
            nc.sync.dma_start(out=gmaxr[0:1, 0:128], in_=cc_out[None, :])
            nc.vector.tensor_reduce(
                out=sg[:, :], in_=gmaxr[:, :], axis=mybir.AxisListType.X,
                op=mybir.AluOpType.max,
            )
            nc.vector.reciprocal(inv[:, :], sg[:, :])
            nc.vector.tensor_scalar_mul(cqi[:, :], inv[:, :], MAXV)
            nc.vector.tensor_mul(dqi[:, :], sg[:, :], sw_[:, :])
            nc.vector.tensor_scalar_mul(dqi[:, :], dqi[:, :], 1.0 / (MAXV * MAXV))
            nc.vector.tensor_copy(bc_in[:, 0:1], cqi[:, :])
            nc.vector.tensor_copy(bc_in[:, 1:2], dqi[:, :])
            bps2 = psx.tile([128, 2], F32, tag="bcast")
            nc.tensor.matmul(bps2[:, 0:2], ones_l[:, :], bc_in[:, 0:2])
            nc.vector.tensor_copy(bvec[:, 0:2], bps2[:, 0:2])
            cq_ap = bvec[:, 0:1]
            dq_ap = bvec[:, 1:2]

            # ---------------- pass 2 ----------------------------------
            qx_tiles = {}

            def quantize_group(gq):
                t = tp.tile([128, QG * W], F16)
                nc.scalar.activation(
                    out=t[:, :], in_=x4[:, gq * QG * W : (gq + 1) * QG * W],
                    func=mybir.ActivationFunctionType.Copy,
                    scale=cq_ap, bias=RND,
                )
                qt = qxp.tile([128, QG * W], BF16)
                with nc.allow_low_precision("int8 values exact in fp16"):
                    nc.vector.tensor_scalar_add(qt[:, :], t[:, :], -RND)
                qx_tiles[gq] = qt

            quantize_group(0)
            if NQG > 1:
                quantize_group(1)

            ot = None
            for q in range(NQ):
                gq, off = divmod(q, QG)
                off *= W
                if q % QG == 0 and gq + 2 <= NQG - 1:
                    quantize_group(gq + 2)

                cur = qx_tiles[gq]
                if q == 0:
                    prev_t = None
                elif q % QG:
                    prev_t, prev_off = cur, off - W
                else:
                    prev_t, prev_off = qx_tiles[gq - 1], (QG - 1) * W
                if q == NQ - 1:
                    next_t = None
                elif (q % QG) != QG - 1:
                    next_t, next_off = cur, off + W
                else:
                    next_t, next_off = qx_tiles[gq + 1], 0

                pst = psp.tile([128, W], F32, tag="pst")
                # (lhsT, rhs, out); kw order 1,0,2; boundary sp/sn interleaved
                mms = [
                    (w4[:, 128:256], cur[:, off : off + 512], pst[:, 0:512],
                     (0, 0)),
                    (w4[:, 0:128], cur[:, off : off + 511], pst[:, 1:512],
                     (0, 0)),
                    (w4[:, 256:384], cur[:, off + 1 : off + 512],
                     pst[:, 0:511], (0, 0)),
                ]
                bnd = []
                if prev_t is not None:
                    p0 = prev_off
                    bnd.append([
                        (bw[96:128, 32:64], prev_t[96:128, p0 : p0 + 512],
                         pst[0:32, 0:512], (96, 0)),
                        (bw[96:128, 0:32], prev_t[96:128, p0 : p0 + 511],
                         pst[0:32, 1:512], (96, 0)),
                        (bw[96:128, 64:96], prev_t[96:128, p0 + 1 : p0 + 512],
                         pst[0:32, 0:511], (96, 0)),
                    ])
                if next_t is not None:
                    n0 = next_off
                    bnd.append([
                        (bw[0:32, 32:64], next_t[0:32, n0 : n0 + 512],
                         pst[96:128, 0:512], (0, 96)),
                        (bw[0:32, 0:32], next_t[0:32, n0 : n0 + 511],
                         pst[96:128, 1:512], (0, 96)),
                        (bw[0:32, 64:96], next_t[0:32, n0 + 1 : n0 + 512],
                         pst[96:128, 0:511], (0, 96)),
                    ])
                # interleave sp/sn per-kw so the (row 96, col 0) and
                # (row 0, col 3) subarray pairs run concurrently
                for trip in zip(*bnd):
                    mms.extend(trip)
                for mi, (lhsT, rhs, outap, tpos) in enumerate(mms):
                    nc.tensor.matmul(
                        outap, lhsT, rhs,
                        start=(mi == 0), stop=(mi == len(mms) - 1),
                        tile_position=tpos,
                    )

                ot = outp.tile([128, W], F16)
                nc.vector.tensor_scalar(
                    out=ot[:, :], in0=pst[:, :],
                    scalar1=dq_ap, scalar2=bias_sb[:, 0:1],
                    op0=mybir.AluOpType.mult, op1=mybir.AluOpType.add,
                )
                ov = out_ext[:, 4 * q : 4 * q + 4, :].rearrange(
                    "o hm w -> hm o w"
                )
                eng = (nc.gpsimd, nc.sync)[q % 2]
                eng.dma_start(out=ov, in_=ot[:, :])

    nc.finalize()
    return nc


_NC_CACHE = {}


def make_in_maps(x, weight, bias):
    x = np.ascontiguousarray(x, dtype=np.float32)
    wt = np.ascontiguousarray(
        np.asarray(weight, dtype=np.float32).transpose(1, 2, 3, 0).reshape(C, -1)
    )
    bias4 = np.ascontiguousarray(
        np.tile(np.asarray(bias, dtype=np.float32), 4)
    )
    return [
        {"x": x[i], "wt": wt, "bias4": bias4} for i in range(N_CORES)
    ]


def kernel(x, weight, bias):
    if "nc" not in _NC_CACHE:
        _NC_CACHE["nc"] = build_nc()
    nc = _NC_CACHE["nc"]
    in_maps = make_in_maps(x, weight, bias)
    res = run_bass_kernel_spmd(nc, in_maps, core_ids=list(range(N_CORES)))
    outs = [
        np.asarray(res.results[i]["out"], dtype=np.float32)
        for i in range(N_CORES)
    ]
    return np.stack(outs, axis=0)


if __name__ == "__main__":
    build_nc(h=32)
    print("build ok")


# revision 10
# speedup vs baseline: 1.8293x; 1.1652x over previous
"""Quantized int8 conv2d (brevitas-style) on 8 TRN2 NeuronCores.

Data-parallel over batch (1 image / core). Per-tensor symmetric int8
quantization: local abs-max -> AllReduce(max) -> quantize -> 3x3 conv
(stride 1, pad 1) as PE col-tiled matmuls -> dequant + bias.

Design notes:
- Pass 1: plain fp32 SWDGE loads (16x 2MB, 4 per-hm 3-dim DMAs each,
  spread over all 16 SDMA engines), scalar converts to the fp16 SBUF
  cache, vector abs-max per group rides on the staging tile.
- round(v) = fp16(v + 1536) - 1536 (fp16 RNE at the [1024,2048) binade
  has ulp=1 -> exact round-half-even). The subtract writes bf16 (ints
  <= 127 are exact) so matmuls run at the PE's full bf16 rate, and
  zero-padding at image borders needs no correction matmuls.
- Conv: output rows grouped by 4 (c = h%4), partition = 32*(h%4)+ch.
  Per block: 3 full K=128 matmuls (one per kw; h-taps folded into the
  (hm -> c) block structure of the lhsT) + up to 6 K=32 boundary
  matmuls that read the adjacent block's edge row DIRECTLY from its
  SBUF tile (partitions 96-127 / 0-31) via PE row/col tile placement
  (no staging DMA; sp/sn pairs run on disjoint 32x32 subarrays).
- Blocks are processed in interleaved groups of 3 (6 PSUM banks) so
  each main lhsT is loaded once per 3 matmuls: fewer LDWEIGHTS stalls
  keeps the PE activity dense enough for HAM to unthrottle to 2.4 GHz.
- fp16 output (halves store traffic; well within 2e-2), upcast on host.
  Stores alternate gpsimd/sync queues (never scalar, which quantizes).
- weight comes in host-pre-transposed as wt[i, (kh kw o)] so the SBUF
  replica loads are contiguous; bias comes host-replicated x4.
"""

import sys

if "/opt/trn_rl_repo" not in sys.path:
    sys.path.insert(0, "/opt/trn_rl_repo")

import numpy as np

import concourse.bass as bass
import concourse.bacc as bacc
import concourse.mybir as mybir
from concourse import tile
from concourse.bass_utils import run_bass_kernel_spmd

N_CORES = 8
C = 32
O = 32
H = 512
W = 512
F32 = mybir.dt.float32
F16 = mybir.dt.float16
BF16 = mybir.dt.bfloat16

MAXV = 127.0
RND = 1536.0


def build_nc(h=H):
    nc = bacc.Bacc(None, target_bir_lowering=False, debug=False)
    NQ = h // 4          # 4-row blocks
    LG = min(8, NQ)      # blocks per load group (8 -> 2MB loads)
    NLG = NQ // LG
    QG = min(4, NQ)      # blocks per quantize group
    NQG = NQ // QG
    BI = 3               # block interleave (shares lhsT across BI matmuls)

    x_ext = nc.declare_dram_parameter("x", [C, h, W], F32, isOutput=False)
    # wt = weight.transpose(1,2,3,0).reshape(C,-1), prepared on host
    w_ext = nc.declare_dram_parameter("wt", [C, 3 * 3 * O], F32, isOutput=False)
    b_ext = nc.declare_dram_parameter("bias4", [128], F32, isOutput=False)
    out_ext = nc.declare_dram_parameter("out", [O, h, W], F16, isOutput=True)

    cc_in = nc.dram_tensor("cc_in", [128], F32)
    cc_out = nc.dram_tensor("cc_out", [128], F32, addr_space="Shared")

    with tile.TileContext(nc) as tc:
        with (
            tc.tile_pool(name="persist", bufs=1) as persist,
            tc.tile_pool(name="stg", bufs=2) as stgp,
            tc.tile_pool(name="tp", bufs=2) as tp,
            tc.tile_pool(name="qx", bufs=4) as qxp,
            tc.tile_pool(name="outp", bufs=6) as outp,
            tc.tile_pool(name="psp", bufs=6, space="PSUM") as psp,
            tc.tile_pool(name="psx", bufs=2, space="PSUM") as psx,
        ):
            # ---------------- persistent SBUF tensors ----------------
            x4 = persist.tile([128, NQ * W], F16)
            maxes = persist.tile([128, NLG], F32)
            wsb = persist.tile([128, 288], F32)
            tq = persist.tile([128, 288], F16)
            qw = persist.tile([128, 288], BF16)
            w4 = persist.tile([128, 3 * 128], BF16)  # main lhsT: kw blocks (c,o)
            bw = persist.tile([128, 96], BF16)       # boundary lhsT
            ones_l = persist.tile([1, 128], F32)
            bias_sb = persist.tile([128, 1], F32)
            gmax = persist.tile([128, 1], F32)
            gmaxr = persist.tile([1, 128], F32)
            wred = persist.tile([128, 1], F32)
            wredr = persist.tile([1, 128], F32)
            sg = persist.tile([1, 1], F32)
            sw_ = persist.tile([1, 1], F32)
            inv = persist.tile([1, 1], F32)
            invw = persist.tile([1, 1], F32)
            cwi = persist.tile([1, 1], F32)
            cqi = persist.tile([1, 1], F32)
            dqi = persist.tile([1, 1], F32)
            bc_in = persist.tile([1, 2], F32)
            bvec = persist.tile([128, 2], F32)
            cw_ap = persist.tile([128, 1], F32)

            # ---------------- weight path (local) --------------------
            for g in range(4):
                eng = (nc.sync, nc.scalar)[g % 2]
                eng.dma_start(out=wsb[32 * g : 32 * g + 32, :], in_=w_ext[:, :])
            nc.sync.dma_start(out=bias_sb[:, :], in_=b_ext[:, None])
            nc.gpsimd.memset(ones_l[:, :], 1.0)
            nc.gpsimd.memset(w4[:, :], 0.0)
            nc.gpsimd.memset(bw[:, :], 0.0)

            # sw = max |w| (X-reduce, fold partitions to a row, reduce again)
            nc.vector.tensor_reduce(
                out=wred[:, :], in_=wsb[:, :], axis=mybir.AxisListType.X,
                op=mybir.AluOpType.max, apply_absolute_value=True,
            )
            nc.sync.dma_start(out=wredr[0:1, 0:128], in_=wred[:, 0:1])
            nc.vector.tensor_reduce(
                out=sw_[:, :], in_=wredr[:, :], axis=mybir.AxisListType.X,
                op=mybir.AluOpType.max,
            )
            nc.vector.reciprocal(invw[:, :], sw_[:, :])
            nc.vector.tensor_scalar_mul(cwi[:, :], invw[:, :], MAXV)

            bps = psx.tile([128, 2], F32, tag="bcast")
            nc.tensor.matmul(bps[:, 0:1], ones_l[:, :], cwi[:, :])
            nc.vector.tensor_copy(cw_ap[:, :], bps[:, 0:1])

            # qw = round(w * 127/sw) via fp16 +1536 trick, then remove offset
            nc.scalar.activation(
                out=tq[:, :], in_=wsb[:, :],
                func=mybir.ActivationFunctionType.Copy,
                scale=cw_ap[:, 0:1], bias=RND,
            )
            with nc.allow_low_precision("int8 values exact in fp16/bf16"):
                nc.vector.tensor_scalar_add(qw[:, :], tq[:, :], -RND)
                # main lhsT: w4[32*hm+i, kw*128+c*32+o] = qw[o,i,hm-c+1,kw]
                for cix in range(4):
                    for kw in range(3):
                        for kh in range(3):
                            hm = cix + kh - 1
                            if not (0 <= hm <= 3):
                                continue
                            nc.vector.tensor_copy(
                                w4[32 * hm : 32 * hm + 32,
                                   kw * 128 + cix * 32 : kw * 128 + cix * 32 + 32],
                                qw[0:32, kh * 96 + kw * 32 : kh * 96 + kw * 32 + 32],
                            )
                # boundary lhsT:
                #   bw[96+i, kw*32+o] = qw[o,i,kh=0,kw]  (sp: prev block row 3)
                #   bw[i,    kw*32+o] = qw[o,i,kh=2,kw]  (sn: next block row 0)
                for kw in range(3):
                    nc.vector.tensor_copy(
                        bw[96:128, kw * 32 : kw * 32 + 32],
                        qw[96:128, 0 * 96 + kw * 32 : 0 * 96 + kw * 32 + 32],
                    )
                    nc.vector.tensor_copy(
                        bw[0:32, kw * 32 : kw * 32 + 32],
                        qw[0:32, 2 * 96 + kw * 32 : 2 * 96 + kw * 32 + 32],
                    )

            # ------------- pass 1: load fp32, convert, absmax ---------
            for g in range(NLG):
                stg = stgp.tile([128, LG * W], F32)
                for hm in range(4):
                    xv = x_ext[
                        :, 4 * LG * g + hm : 4 * LG * (g + 1) : 4, :
                    ]
                    nc.gpsimd.dma_start(
                        out=stg[32 * hm : 32 * hm + 32, :].rearrange(
                            "p (j w) -> p j w", j=LG
                        ),
                        in_=xv.rearrange("i j w -> i j w"),
                    )
                nc.scalar.activation(
                    out=x4[:, g * LG * W : (g + 1) * LG * W], in_=stg[:, :],
                    func=mybir.ActivationFunctionType.Copy,
                )
                nc.vector.tensor_reduce(
                    out=maxes[:, g : g + 1], in_=stg[:, :],
                    axis=mybir.AxisListType.X,
                    op=mybir.AluOpType.max, apply_absolute_value=True,
                )
            nc.vector.tensor_reduce(
                out=gmax[:, :], in_=maxes[:, :], axis=mybir.AxisListType.X,
                op=mybir.AluOpType.max,
            )

            # ------------- all-reduce(max) across 8 cores -------------
            nc.sync.dma_start(out=cc_in[:, None], in_=gmax[:, :])
            nc.gpsimd.collective_compute(
                "AllReduce", mybir.AluOpType.max,
                replica_groups=[list(range(N_CORES))],
                ins=[cc_in[:].opt()], outs=[cc_out[:].opt()],
            )
            nc.sync.dma_start(out=gmaxr[0:1, 0:128], in_=cc_out[None, :])
            nc.vector.tensor_reduce(
                out=sg[:, :], in_=gmaxr[:, :], axis=mybir.AxisListType.X,
                op=mybir.AluOpType.max,
            )
            nc.vector.reciprocal(inv[:, :], sg[:, :])
            nc.vector.tensor_scalar_mul(cqi[:, :], inv[:, :], MAXV)
            nc.vector.tensor_mul(dqi[:, :], sg[:, :], sw_[:, :])
            nc.vector.tensor_scalar_mul(dqi[:, :], dqi[:, :], 1.0 / (MAXV * MAXV))
            nc.vector.tensor_copy(bc_in[:, 0:1], cqi[:, :])
            nc.vector.tensor_copy(bc_in[:, 1:2], dqi[:, :])
            bps2 = psx.tile([128, 2], F32, tag="bcast")
            nc.tensor.matmul(bps2[:, 0:2], ones_l[:, :], bc_in[:, 0:2])
            nc.vector.tensor_copy(bvec[:, 0:2], bps2[:, 0:2])
            cq_ap = bvec[:, 0:1]
            dq_ap = bvec[:, 1:2]

            # ---------------- pass 2 ----------------------------------
            qx_tiles = {}

            def quantize_group(gq):
                if gq in qx_tiles or gq >= NQG:
                    return
                t = tp.tile([128, QG * W], F16)
                nc.scalar.activation(
                    out=t[:, :], in_=x4[:, gq * QG * W : (gq + 1) * QG * W],
                    func=mybir.ActivationFunctionType.Copy,
                    scale=cq_ap, bias=RND,
                )
                qt = qxp.tile([128, QG * W], BF16)
                with nc.allow_low_precision("int8 values exact in bf16"):
                    nc.vector.tensor_scalar_add(qt[:, :], t[:, :], -RND)
                qx_tiles[gq] = qt

            def neighbors(q):
                gq, off = divmod(q, QG)
                off *= W
                cur = qx_tiles[gq]
                if q == 0:
                    prev = None
                elif q % QG:
                    prev = (cur, off - W)
                else:
                    prev = (qx_tiles[gq - 1], (QG - 1) * W)
                if q == NQ - 1:
                    nxt = None
                elif (q % QG) != QG - 1:
                    nxt = (cur, off + W)
                else:
                    nxt = (qx_tiles[gq + 1], 0)
                return cur, off, prev, nxt

            quantize_group(0)
            quantize_group(1)

            for q0 in range(0, NQ, BI):
                qs = list(range(q0, min(q0 + BI, NQ)))
                # prefetch quantize ~2 interleave-groups ahead
                for qq in range(q0, min(q0 + 3 * BI + 1, NQ)):
                    quantize_group(qq // QG)

                psts = {}
                mains = {}
                bounds = {}
                for q in qs:
                    pst = psp.tile([128, W], F32, tag="pst")
                    psts[q] = pst
                    cur, off, prev, nxt = neighbors(q)
                    mains[q] = [
                        (w4[:, 128:256], cur[:, off : off + 512],
                         pst[:, 0:512], (0, 0)),
                        (w4[:, 0:128], cur[:, off : off + 511],
                         pst[:, 1:512], (0, 0)),
                        (w4[:, 256:384], cur[:, off + 1 : off + 512],
                         pst[:, 0:511], (0, 0)),
                    ]
                    bnd = []
                    if prev is not None:
                        pt, p0 = prev
                        bnd.append([
                            (bw[96:128, 32:64], pt[96:128, p0 : p0 + 512],
                             pst[0:32, 0:512], (96, 0)),
                            (bw[96:128, 0:32], pt[96:128, p0 : p0 + 511],
                             pst[0:32, 1:512], (96, 0)),
                            (bw[96:128, 64:96], pt[96:128, p0 + 1 : p0 + 512],
                             pst[0:32, 0:511], (96, 0)),
                        ])
                    if nxt is not None:
                        nt, n0 = nxt
                        bnd.append([
                            (bw[0:32, 32:64], nt[0:32, n0 : n0 + 512],
                             pst[96:128, 0:512], (0, 96)),
                            (bw[0:32, 0:32], nt[0:32, n0 : n0 + 511],
                             pst[96:128, 1:512], (0, 96)),
                            (bw[0:32, 64:96], nt[0:32, n0 + 1 : n0 + 512],
                             pst[96:128, 0:511], (0, 96)),
                        ])
                    # per-kw rounds: each round holds the sp/sn pair (or
                    # single at image edges) that runs concurrently on
                    # disjoint 32x32 subarrays
                    bounds[q] = list(zip(*bnd)) if bnd else []

                # emission order: same main lhsT across the BI blocks
                # back-to-back (one LDWEIGHTS per BI matmuls), then the
                # cheap-LDW boundary matmuls, kw-round-major with each
                # block's sp/sn pair adjacent.
                order = []
                for idx in range(3):
                    for q in qs:
                        order.append((q, mains[q][idx]))
                for idx in range(3):
                    for q in qs:
                        for mm in bounds[q][idx] if idx < len(bounds[q]) else ():
                            order.append((q, mm))
                counts = {q: 0 for q in qs}
                totals = {
                    q: len(mains[q]) + sum(len(r) for r in bounds[q])
                    for q in qs
                }
                for q, (lhsT, rhs, outap, tpos) in order:
                    counts[q] += 1
                    nc.tensor.matmul(
                        outap, lhsT, rhs,
                        start=(counts[q] == 1), stop=(counts[q] == totals[q]),
                        tile_position=tpos,
                    )

                for q in qs:
                    ot = outp.tile([128, W], F16)
                    nc.vector.tensor_scalar(
                        out=ot[:, :], in0=psts[q][:, :],
                        scalar1=dq_ap, scalar2=bias_sb[:, 0:1],
                        op0=mybir.AluOpType.mult, op1=mybir.AluOpType.add,
                    )
                    ov = out_ext[:, 4 * q : 4 * q + 4, :].rearrange(
                        "o hm w -> hm o w"
                    )
                    eng = (nc.gpsimd, nc.sync)[q % 2]
                    eng.dma_start(out=ov, in_=ot[:, :])

    nc.finalize()
    return nc


_NC_CACHE = {}


def make_in_maps(x, weight, bias):
    x = np.ascontiguousarray(x, dtype=np.float32)
    wt = np.ascontiguousarray(
        np.asarray(weight, dtype=np.float32).transpose(1, 2, 3, 0).reshape(C, -1)
    )
    bias4 = np.ascontiguousarray(
        np.tile(np.asarray(bias, dtype=np.float32), 4)
    )
    return [
        {"x": x[i], "wt": wt, "bias4": bias4} for i in range(N_CORES)
    ]


def kernel(x, weight, bias):
    if "nc" not in _NC_CACHE:
        _NC_CACHE["nc"] = build_nc()
    nc = _NC_CACHE["nc"]
    in_maps = make_in_maps(x, weight, bias)
    res = run_bass_kernel_spmd(nc, in_maps, core_ids=list(range(N_CORES)))
    outs = [
        np.asarray(res.results[i]["out"], dtype=np.float32)
        for i in range(N_CORES)
    ]
    return np.stack(outs, axis=0)


if __name__ == "__main__":
    build_nc(h=32)
    print("build ok")


# revision 11
# speedup vs baseline: 2.0088x; 1.0981x over previous
"""Quantized int8 conv2d (brevitas-style) on 8 TRN2 NeuronCores.

Data-parallel over batch (1 image / core). Per-tensor symmetric int8
quantization: local abs-max -> AllReduce(max) -> quantize -> 3x3 conv
(stride 1, pad 1) as PE col-tiled matmuls -> dequant + bias.

Design notes:
- Pass 1: plain fp32 SWDGE loads (16x 2MB, 4 per-hm 3-dim DMAs each,
  spread over all 16 SDMA engines), scalar converts to the fp16 SBUF
  cache, vector abs-max per group rides on the staging tile.
- round(v) = fp16(v + 1536) - 1536 (fp16 RNE at the [1024,2048) binade
  has ulp=1 -> exact round-half-even). The subtract writes bf16 (ints
  <= 127 are exact) so matmuls run at the PE's full bf16 rate, and
  zero-padding at image borders needs no correction matmuls.
- Conv: output rows grouped by 4 (c = h%4), partition = 32*(h%4)+ch.
  Per block: 3 full K=128 matmuls (one per kw; h-taps folded into the
  (hm -> c) block structure of the lhsT) + up to 6 K=32 boundary
  matmuls that read the adjacent block's edge row DIRECTLY from its
  SBUF tile (partitions 96-127 / 0-31) via PE row/col tile placement
  (no staging DMA; sp/sn pairs run on disjoint 32x32 subarrays).
- Blocks are processed in interleaved groups of 3 (6 PSUM banks) so
  each main lhsT is loaded once per 3 matmuls: fewer LDWEIGHTS stalls
  keeps the PE activity dense enough for HAM to unthrottle to 2.4 GHz.
- fp16 output (halves store traffic; well within 2e-2), upcast on host.
  Stores alternate gpsimd/sync queues (never scalar, which quantizes).
- weight comes in host-pre-transposed as wt[i, (kh kw o)] so the SBUF
  replica loads are contiguous; bias comes host-replicated x4.
"""

import sys

if "/opt/trn_rl_repo" not in sys.path:
    sys.path.insert(0, "/opt/trn_rl_repo")

import numpy as np

import concourse.bass as bass
import concourse.bacc as bacc
import concourse.mybir as mybir
from concourse import tile
from concourse.bass_utils import run_bass_kernel_spmd

N_CORES = 8
C = 32
O = 32
H = 512
W = 512
F32 = mybir.dt.float32
F16 = mybir.dt.float16
BF16 = mybir.dt.bfloat16

MAXV = 127.0
RND = 1536.0


def build_nc(h=H):
    nc = bacc.Bacc(None, target_bir_lowering=False, debug=False)
    NQ = h // 4          # 4-row blocks
    LG = min(16, NQ)     # blocks per load group (16 -> 4MB cast loads)
    NLG = NQ // LG
    QG = min(4, NQ)      # blocks per quantize group
    NQG = NQ // QG
    BI = 3               # block interleave (shares lhsT across BI matmuls)

    x_ext = nc.declare_dram_parameter("x", [C, h, W], F32, isOutput=False)
    # wt = weight.transpose(1,2,3,0).reshape(C,-1), prepared on host
    w_ext = nc.declare_dram_parameter("wt", [C, 3 * 3 * O], F32, isOutput=False)
    b_ext = nc.declare_dram_parameter("bias4", [128], F32, isOutput=False)
    out_ext = nc.declare_dram_parameter("out", [O, h, W], F16, isOutput=True)

    cc_in = nc.dram_tensor("cc_in", [128], F32)
    cc_out = nc.dram_tensor("cc_out", [128], F32, addr_space="Shared")

    with tile.TileContext(nc) as tc:
        with (
            tc.tile_pool(name="persist", bufs=1) as persist,
            tc.tile_pool(name="tp", bufs=3) as tp,
            tc.tile_pool(name="qx", bufs=5) as qxp,
            tc.tile_pool(name="outp", bufs=8) as outp,
            tc.tile_pool(name="psp", bufs=6, space="PSUM") as psp,
            tc.tile_pool(name="psx", bufs=1, space="PSUM") as psx,
        ):
            # ---------------- persistent SBUF tensors ----------------
            x4 = persist.tile([128, NQ * W], F16)
            maxes = persist.tile([128, NQG], F32)
            wsb = persist.tile([128, 288], F32)
            tq = persist.tile([128, 288], F16)
            qw = persist.tile([128, 288], BF16)
            w4 = persist.tile([128, 3 * 128], BF16)  # main lhsT: kw blocks (c,o)
            bw = persist.tile([128, 96], BF16)       # boundary lhsT
            ones_l = persist.tile([1, 128], F32)
            bias_sb = persist.tile([128, 1], F32)
            gmax = persist.tile([128, 1], F32)
            gmaxr = persist.tile([1, 128], F32)
            wred = persist.tile([128, 1], F32)
            wredr = persist.tile([1, 128], F32)
            sg = persist.tile([1, 1], F32)
            sw_ = persist.tile([1, 1], F32)
            inv = persist.tile([1, 1], F32)
            invw = persist.tile([1, 1], F32)
            cwi = persist.tile([1, 1], F32)
            cqi = persist.tile([1, 1], F32)
            dqi = persist.tile([1, 1], F32)
            bc_in = persist.tile([1, 2], F32)
            bvec = persist.tile([128, 2], F32)
            cw_ap = persist.tile([128, 1], F32)

            # ---------------- weight path (local) --------------------
            for g in range(4):
                eng = (nc.sync, nc.scalar)[g % 2]
                eng.dma_start(out=wsb[32 * g : 32 * g + 32, :], in_=w_ext[:, :])
            nc.sync.dma_start(out=bias_sb[:, :], in_=b_ext[:, None])
            nc.gpsimd.memset(ones_l[:, :], 1.0)
            nc.gpsimd.memset(w4[:, :], 0.0)
            nc.gpsimd.memset(bw[:, :], 0.0)

            # sw = max |w| (X-reduce, fold partitions to a row, reduce again)
            nc.vector.tensor_reduce(
                out=wred[:, :], in_=wsb[:, :], axis=mybir.AxisListType.X,
                op=mybir.AluOpType.max, apply_absolute_value=True,
            )
            nc.sync.dma_start(out=wredr[0:1, 0:128], in_=wred[:, 0:1])
            nc.vector.tensor_reduce(
                out=sw_[:, :], in_=wredr[:, :], axis=mybir.AxisListType.X,
                op=mybir.AluOpType.max,
            )
            nc.vector.reciprocal(invw[:, :], sw_[:, :])
            nc.vector.tensor_scalar_mul(cwi[:, :], invw[:, :], MAXV)

            bps = psx.tile([128, 2], F32, tag="bcast")
            nc.tensor.matmul(bps[:, 0:1], ones_l[:, :], cwi[:, :])
            nc.vector.tensor_copy(cw_ap[:, :], bps[:, 0:1])

            # qw = round(w * 127/sw) via fp16 +1536 trick, then remove offset
            nc.scalar.activation(
                out=tq[:, :], in_=wsb[:, :],
                func=mybir.ActivationFunctionType.Copy,
                scale=cw_ap[:, 0:1], bias=RND,
            )
            with nc.allow_low_precision("int8 values exact in fp16/bf16"):
                nc.vector.tensor_scalar_add(qw[:, :], tq[:, :], -RND)
                # main lhsT: w4[32*hm+i, kw*128+c*32+o] = qw[o,i,hm-c+1,kw]
                for cix in range(4):
                    for kw in range(3):
                        for kh in range(3):
                            hm = cix + kh - 1
                            if not (0 <= hm <= 3):
                                continue
                            nc.vector.tensor_copy(
                                w4[32 * hm : 32 * hm + 32,
                                   kw * 128 + cix * 32 : kw * 128 + cix * 32 + 32],
                                qw[0:32, kh * 96 + kw * 32 : kh * 96 + kw * 32 + 32],
                            )
                # boundary lhsT:
                #   bw[96+i, kw*32+o] = qw[o,i,kh=0,kw]  (sp: prev block row 3)
                #   bw[i,    kw*32+o] = qw[o,i,kh=2,kw]  (sn: next block row 0)
                for kw in range(3):
                    nc.vector.tensor_copy(
                        bw[96:128, kw * 32 : kw * 32 + 32],
                        qw[96:128, 0 * 96 + kw * 32 : 0 * 96 + kw * 32 + 32],
                    )
                    nc.vector.tensor_copy(
                        bw[0:32, kw * 32 : kw * 32 + 32],
                        qw[0:32, 2 * 96 + kw * 32 : 2 * 96 + kw * 32 + 32],
                    )

            # ------- pass 1: cast-DMA x to fp16 SBUF cache + absmax ---
            # 4 per-hm 3-dim DMAs per 4MB group (fp32->fp16 inline cast,
            # SWDGE); vector abs-max in QG-block chunks pipelines behind.
            for g in range(NLG):
                for hm in range(4):
                    xv = x_ext[
                        :, 4 * LG * g + hm : 4 * LG * (g + 1) : 4, :
                    ]
                    nc.gpsimd.dma_start(
                        out=x4[32 * hm : 32 * hm + 32,
                               g * LG * W : (g + 1) * LG * W].rearrange(
                            "p (j w) -> p j w", j=LG
                        ),
                        in_=xv.rearrange("i j w -> i j w"),
                    )
                for c0 in range(g * LG * W, (g + 1) * LG * W, QG * W):
                    gq = c0 // (QG * W)
                    nc.vector.tensor_reduce(
                        out=maxes[:, gq : gq + 1],
                        in_=x4[:, c0 : c0 + QG * W],
                        axis=mybir.AxisListType.X,
                        op=mybir.AluOpType.max, apply_absolute_value=True,
                    )
            nc.vector.tensor_reduce(
                out=gmax[:, :], in_=maxes[:, :], axis=mybir.AxisListType.X,
                op=mybir.AluOpType.max,
            )

            # ------------- all-reduce(max) across 8 cores -------------
            nc.sync.dma_start(out=cc_in[:, None], in_=gmax[:, :])
            nc.gpsimd.collective_compute(
                "AllReduce", mybir.AluOpType.max,
                replica_groups=[list(range(N_CORES))],
                ins=[cc_in[:].opt()], outs=[cc_out[:].opt()],
            )
            nc.sync.dma_start(out=gmaxr[0:1, 0:128], in_=cc_out[None, :])
            nc.vector.tensor_reduce(
                out=sg[:, :], in_=gmaxr[:, :], axis=mybir.AxisListType.X,
                op=mybir.AluOpType.max,
            )
            nc.vector.reciprocal(inv[:, :], sg[:, :])
            nc.vector.tensor_scalar_mul(cqi[:, :], inv[:, :], MAXV)
            nc.vector.tensor_mul(dqi[:, :], sg[:, :], sw_[:, :])
            nc.vector.tensor_scalar_mul(dqi[:, :], dqi[:, :], 1.0 / (MAXV * MAXV))
            nc.vector.tensor_copy(bc_in[:, 0:1], cqi[:, :])
            nc.vector.tensor_copy(bc_in[:, 1:2], dqi[:, :])
            bps2 = psx.tile([128, 2], F32, tag="bcast")
            nc.tensor.matmul(bps2[:, 0:2], ones_l[:, :], bc_in[:, 0:2])
            nc.vector.tensor_copy(bvec[:, 0:2], bps2[:, 0:2])
            cq_ap = bvec[:, 0:1]
            dq_ap = bvec[:, 1:2]

            # ---------------- pass 2 ----------------------------------
            qx_tiles = {}

            def quantize_group(gq):
                if gq in qx_tiles or gq >= NQG:
                    return
                t = tp.tile([128, QG * W], F16)
                nc.scalar.activation(
                    out=t[:, :], in_=x4[:, gq * QG * W : (gq + 1) * QG * W],
                    func=mybir.ActivationFunctionType.Copy,
                    scale=cq_ap, bias=RND,
                )
                qt = qxp.tile([128, QG * W], BF16)
                with nc.allow_low_precision("int8 values exact in bf16"):
                    nc.vector.tensor_scalar_add(qt[:, :], t[:, :], -RND)
                qx_tiles[gq] = qt

            def neighbors(q):
                gq, off = divmod(q, QG)
                off *= W
                cur = qx_tiles[gq]
                if q == 0:
                    prev = None
                elif q % QG:
                    prev = (cur, off - W)
                else:
                    prev = (qx_tiles[gq - 1], (QG - 1) * W)
                if q == NQ - 1:
                    nxt = None
                elif (q % QG) != QG - 1:
                    nxt = (cur, off + W)
                else:
                    nxt = (qx_tiles[gq + 1], 0)
                return cur, off, prev, nxt

            quantize_group(0)
            quantize_group(1)

            for q0 in range(0, NQ, BI):
                qs = list(range(q0, min(q0 + BI, NQ)))
                # prefetch quantize ~2 interleave-groups ahead
                for qq in range(q0, min(q0 + 3 * BI + 1, NQ)):
                    quantize_group(qq // QG)

                psts = {}
                mains = {}
                bounds = {}
                for q in qs:
                    pst = psp.tile([128, W], F32, tag="pst")
                    psts[q] = pst
                    cur, off, prev, nxt = neighbors(q)
                    mains[q] = [
                        (w4[:, 128:256], cur[:, off : off + 512],
                         pst[:, 0:512], (0, 0)),
                        (w4[:, 0:128], cur[:, off : off + 511],
                         pst[:, 1:512], (0, 0)),
                        (w4[:, 256:384], cur[:, off + 1 : off + 512],
                         pst[:, 0:511], (0, 0)),
                    ]
                    bnd = []
                    if prev is not None:
                        pt, p0 = prev
                        bnd.append([
                            (bw[96:128, 32:64], pt[96:128, p0 : p0 + 512],
                             pst[0:32, 0:512], (96, 0)),
                            (bw[96:128, 0:32], pt[96:128, p0 : p0 + 511],
                             pst[0:32, 1:512], (96, 0)),
                            (bw[96:128, 64:96], pt[96:128, p0 + 1 : p0 + 512],
                             pst[0:32, 0:511], (96, 0)),
                        ])
                    if nxt is not None:
                        nt, n0 = nxt
                        bnd.append([
                            (bw[0:32, 32:64], nt[0:32, n0 : n0 + 512],
                             pst[96:128, 0:512], (0, 96)),
                            (bw[0:32, 0:32], nt[0:32, n0 : n0 + 511],
                             pst[96:128, 1:512], (0, 96)),
                            (bw[0:32, 64:96], nt[0:32, n0 + 1 : n0 + 512],
                             pst[96:128, 0:511], (0, 96)),
                        ])
                    # per-kw rounds: each round holds the sp/sn pair (or
                    # single at image edges) that runs concurrently on
                    # disjoint 32x32 subarrays
                    bounds[q] = list(zip(*bnd)) if bnd else []

                # emission order: same main lhsT across the BI blocks
                # back-to-back (one LDWEIGHTS per BI matmuls), then the
                # cheap-LDW boundary matmuls, kw-round-major with each
                # block's sp/sn pair adjacent.
                order = []
                for idx in range(3):
                    for q in qs:
                        order.append((q, mains[q][idx]))
                for idx in range(3):
                    for q in qs:
                        for mm in bounds[q][idx] if idx < len(bounds[q]) else ():
                            order.append((q, mm))
                counts = {q: 0 for q in qs}
                totals = {
                    q: len(mains[q]) + sum(len(r) for r in bounds[q])
                    for q in qs
                }
                for q, (lhsT, rhs, outap, tpos) in order:
                    counts[q] += 1
                    nc.tensor.matmul(
                        outap, lhsT, rhs,
                        start=(counts[q] == 1), stop=(counts[q] == totals[q]),
                        tile_position=tpos,
                    )

                for q in qs:
                    ot = outp.tile([128, W], F16)
                    nc.vector.tensor_scalar(
                        out=ot[:, :], in0=psts[q][:, :],
                        scalar1=dq_ap, scalar2=bias_sb[:, 0:1],
                        op0=mybir.AluOpType.mult, op1=mybir.AluOpType.add,
                    )
                    ov = out_ext[:, 4 * q : 4 * q + 4, :].rearrange(
                        "o hm w -> hm o w"
                    )
                    eng = (nc.gpsimd, nc.sync)[q % 2]
                    eng.dma_start(out=ov, in_=ot[:, :])

    nc.finalize()
    return nc


_NC_CACHE = {}


def make_in_maps(x, weight, bias):
    x = np.ascontiguousarray(x, dtype=np.float32)
    wt = np.ascontiguousarray(
        np.asarray(weight, dtype=np.float32).transpose(1, 2, 3, 0).reshape(C, -1)
    )
    bias4 = np.ascontiguousarray(
        np.tile(np.asarray(bias, dtype=np.float32), 4)
    )
    return [
        {"x": x[i], "wt": wt, "bias4": bias4} for i in range(N_CORES)
    ]


def kernel(x, weight, bias):
    if "nc" not in _NC_CACHE:
        _NC_CACHE["nc"] = build_nc()
    nc = _NC_CACHE["nc"]
    in_maps = make_in_maps(x, weight, bias)
    res = run_bass_kernel_spmd(nc, in_maps, core_ids=list(range(N_CORES)))
    outs = [
        np.asarray(res.results[i]["out"], dtype=np.float32)
        for i in range(N_CORES)
    ]
    return np.stack(outs, axis=0)


if __name__ == "__main__":
    build_nc(h=32)
    print("build ok")


# revision 12
# speedup vs baseline: 2.3124x; 1.1511x over previous
"""Quantized int8 conv2d (brevitas-style) on 8 TRN2 NeuronCores.

Data-parallel over batch (1 image / core). Per-tensor symmetric int8
quantization: local abs-max -> AllReduce(max) -> quantize -> 3x3 conv
(stride 1, pad 1) as PE col-tiled matmuls -> dequant + bias.

Design notes:
- Pass 1: plain fp32 SWDGE loads (16x 2MB, 4 per-hm 3-dim DMAs each,
  spread over all 16 SDMA engines), scalar converts to the fp16 SBUF
  cache, vector abs-max per group rides on the staging tile.
- round(v) = fp16(v + 1536) - 1536 (fp16 RNE at the [1024,2048) binade
  has ulp=1 -> exact round-half-even). The subtract writes bf16 (ints
  <= 127 are exact) so matmuls run at the PE's full bf16 rate, and
  zero-padding at image borders needs no correction matmuls.
- Conv: output rows grouped by 4 (c = h%4), partition = 32*(h%4)+ch.
  Per block: 3 full K=128 matmuls (one per kw; h-taps folded into the
  (hm -> c) block structure of the lhsT) + up to 6 K=32 boundary
  matmuls that read the adjacent block's edge row DIRECTLY from its
  SBUF tile (partitions 96-127 / 0-31) via PE row/col tile placement
  (no staging DMA; sp/sn pairs run on disjoint 32x32 subarrays).
- Blocks are processed in interleaved groups of 3 (6 PSUM banks) so
  each main lhsT is loaded once per 3 matmuls: fewer LDWEIGHTS stalls
  keeps the PE activity dense enough for HAM to unthrottle to 2.4 GHz.
- fp16 output (halves store traffic; well within 2e-2), upcast on host.
  Stores alternate gpsimd/sync queues (never scalar, which quantizes).
- weight comes in host-pre-transposed as wt[i, (kh kw o)] so the SBUF
  replica loads are contiguous; bias comes host-replicated x4.
"""

import sys

if "/opt/trn_rl_repo" not in sys.path:
    sys.path.insert(0, "/opt/trn_rl_repo")

import numpy as np

import concourse.bass as bass
import concourse.bacc as bacc
import concourse.mybir as mybir
from concourse import tile
from concourse.bass_utils import run_bass_kernel_spmd

N_CORES = 8
C = 32
O = 32
H = 512
W = 512
F32 = mybir.dt.float32
F16 = mybir.dt.float16
BF16 = mybir.dt.bfloat16

MAXV = 127.0
RND = 1536.0


def build_nc(h=H):
    nc = bacc.Bacc(None, target_bir_lowering=False, debug=False)
    NQ = h // 4          # 4-row blocks
    LG = min(16, NQ)     # blocks per load group (16 -> 4MB cast loads)
    NLG = NQ // LG
    QG = min(4, NQ)      # blocks per quantize group
    NQG = NQ // QG
    BI = 3               # block interleave (shares lhsT across BI matmuls)

    x_ext = nc.declare_dram_parameter("x", [C, h, W], F32, isOutput=False)
    # wt = weight.transpose(1,2,3,0).reshape(C,-1), prepared on host
    w_ext = nc.declare_dram_parameter("wt", [C, 3 * 3 * O], F32, isOutput=False)
    b_ext = nc.declare_dram_parameter("bias4", [128], F32, isOutput=False)
    out_ext = nc.declare_dram_parameter("out", [O, h, W], F16, isOutput=True)

    cc_in = nc.dram_tensor("cc_in", [128], F32)
    cc_out = nc.dram_tensor("cc_out", [128], F32, addr_space="Shared")

    with tile.TileContext(nc) as tc:
        with (
            tc.tile_pool(name="persist", bufs=1) as persist,
            tc.tile_pool(name="tp", bufs=3) as tp,
            tc.tile_pool(name="qx", bufs=5) as qxp,
            tc.tile_pool(name="outp", bufs=8) as outp,
            tc.tile_pool(name="psp", bufs=6, space="PSUM") as psp,
            tc.tile_pool(name="psx", bufs=1, space="PSUM") as psx,
        ):
            # ---------------- persistent SBUF tensors ----------------
            x4 = persist.tile([128, NQ * W], F16)
            maxes = persist.tile([128, NQG], F32)
            wsb = persist.tile([128, 288], F32)
            tq = persist.tile([128, 288], F16)
            qw = persist.tile([128, 288], BF16)
            w4 = persist.tile([128, 3 * 128], BF16)  # main lhsT: kw blocks (c,o)
            bw = persist.tile([128, 96], BF16)       # boundary lhsT
            ones_l = persist.tile([1, 128], F32)
            bias_sb = persist.tile([128, 1], F32)
            gmax = persist.tile([128, 1], F32)
            gmaxr = persist.tile([1, 128], F32)
            wred = persist.tile([128, 1], F32)
            wredr = persist.tile([1, 128], F32)
            sg = persist.tile([1, 1], F32)
            sw_ = persist.tile([1, 1], F32)
            inv = persist.tile([1, 1], F32)
            invw = persist.tile([1, 1], F32)
            cwi = persist.tile([1, 1], F32)
            cqi = persist.tile([1, 1], F32)
            dqi = persist.tile([1, 1], F32)
            bc_in = persist.tile([1, 2], F32)
            bvec = persist.tile([128, 2], F32)
            cw_ap = persist.tile([128, 1], F32)

            # ---------------- weight path (local) --------------------
            for g in range(4):
                eng = (nc.sync, nc.scalar)[g % 2]
                eng.dma_start(out=wsb[32 * g : 32 * g + 32, :], in_=w_ext[:, :])
            nc.sync.dma_start(out=bias_sb[:, :], in_=b_ext[:, None])
            nc.gpsimd.memset(ones_l[:, :], 1.0)
            nc.gpsimd.memset(w4[:, :], 0.0)
            nc.gpsimd.memset(bw[:, :], 0.0)

            # sw = max |w| (X-reduce, fold partitions to a row, reduce again)
            nc.vector.tensor_reduce(
                out=wred[:, :], in_=wsb[:, :], axis=mybir.AxisListType.X,
                op=mybir.AluOpType.max, apply_absolute_value=True,
            )
            nc.sync.dma_start(out=wredr[0:1, 0:128], in_=wred[:, 0:1])
            nc.vector.tensor_reduce(
                out=sw_[:, :], in_=wredr[:, :], axis=mybir.AxisListType.X,
                op=mybir.AluOpType.max,
            )
            nc.vector.reciprocal(invw[:, :], sw_[:, :])
            nc.vector.tensor_scalar_mul(cwi[:, :], invw[:, :], MAXV)

            bps = psx.tile([128, 2], F32, tag="bcast")
            nc.tensor.matmul(bps[:, 0:1], ones_l[:, :], cwi[:, :])
            nc.vector.tensor_copy(cw_ap[:, :], bps[:, 0:1])

            # qw = round(w * 127/sw) via fp16 +1536 trick, then remove offset
            nc.scalar.activation(
                out=tq[:, :], in_=wsb[:, :],
                func=mybir.ActivationFunctionType.Copy,
                scale=cw_ap[:, 0:1], bias=RND,
            )
            with nc.allow_low_precision("int8 values exact in fp16/bf16"):
                nc.vector.tensor_scalar_add(qw[:, :], tq[:, :], -RND)
                # main lhsT: w4[32*hm+i, kw*128+c*32+o] = qw[o,i,hm-c+1,kw]
                for cix in range(4):
                    for kw in range(3):
                        for kh in range(3):
                            hm = cix + kh - 1
                            if not (0 <= hm <= 3):
                                continue
                            nc.vector.tensor_copy(
                                w4[32 * hm : 32 * hm + 32,
                                   kw * 128 + cix * 32 : kw * 128 + cix * 32 + 32],
                                qw[0:32, kh * 96 + kw * 32 : kh * 96 + kw * 32 + 32],
                            )
                # boundary lhsT:
                #   bw[96+i, kw*32+o] = qw[o,i,kh=0,kw]  (sp: prev block row 3)
                #   bw[i,    kw*32+o] = qw[o,i,kh=2,kw]  (sn: next block row 0)
                for kw in range(3):
                    nc.vector.tensor_copy(
                        bw[96:128, kw * 32 : kw * 32 + 32],
                        qw[96:128, 0 * 96 + kw * 32 : 0 * 96 + kw * 32 + 32],
                    )
                    nc.vector.tensor_copy(
                        bw[0:32, kw * 32 : kw * 32 + 32],
                        qw[0:32, 2 * 96 + kw * 32 : 2 * 96 + kw * 32 + 32],
                    )

            # ------- pass 1: cast-DMA x to fp16 SBUF cache + absmax ---
            # 4 per-hm 3-dim DMAs per 4MB group (fp32->fp16 inline cast,
            # SWDGE); vector abs-max in QG-block chunks pipelines behind.
            for g in range(NLG):
                for hm in range(4):
                    xv = x_ext[
                        :, 4 * LG * g + hm : 4 * LG * (g + 1) : 4, :
                    ]
                    nc.gpsimd.dma_start(
                        out=x4[32 * hm : 32 * hm + 32,
                               g * LG * W : (g + 1) * LG * W].rearrange(
                            "p (j w) -> p j w", j=LG
                        ),
                        in_=xv.rearrange("i j w -> i j w"),
                    )
                for c0 in range(g * LG * W, (g + 1) * LG * W, QG * W):
                    gq = c0 // (QG * W)
                    nc.vector.tensor_reduce(
                        out=maxes[:, gq : gq + 1],
                        in_=x4[:, c0 : c0 + QG * W],
                        axis=mybir.AxisListType.X,
                        op=mybir.AluOpType.max, apply_absolute_value=True,
                    )
            nc.vector.tensor_reduce(
                out=gmax[:, :], in_=maxes[:, :], axis=mybir.AxisListType.X,
                op=mybir.AluOpType.max,
            )

            # ------------- all-reduce(max) across 8 cores -------------
            nc.sync.dma_start(out=cc_in[:, None], in_=gmax[:, :])
            nc.gpsimd.collective_compute(
                "AllReduce", mybir.AluOpType.max,
                replica_groups=[list(range(N_CORES))],
                ins=[cc_in[:].opt()], outs=[cc_out[:].opt()],
            )
            nc.sync.dma_start(out=gmaxr[0:1, 0:128], in_=cc_out[None, :])
            nc.vector.tensor_reduce(
                out=sg[:, :], in_=gmaxr[:, :], axis=mybir.AxisListType.X,
                op=mybir.AluOpType.max,
            )
            nc.vector.reciprocal(inv[:, :], sg[:, :])
            nc.vector.tensor_scalar_mul(cqi[:, :], inv[:, :], MAXV)
            nc.vector.tensor_mul(dqi[:, :], sg[:, :], sw_[:, :])
            nc.vector.tensor_scalar_mul(dqi[:, :], dqi[:, :], 1.0 / (MAXV * MAXV))
            nc.vector.tensor_copy(bc_in[:, 0:1], cqi[:, :])
            nc.vector.tensor_copy(bc_in[:, 1:2], dqi[:, :])
            bps2 = psx.tile([128, 2], F32, tag="bcast")
            nc.tensor.matmul(bps2[:, 0:2], ones_l[:, :], bc_in[:, 0:2])
            nc.vector.tensor_copy(bvec[:, 0:2], bps2[:, 0:2])
            cq_ap = bvec[:, 0:1]
            dq_ap = bvec[:, 1:2]

            # ---------------- pass 2 ----------------------------------
            qx_tiles = {}

            def quantize_group(gq):
                if gq in qx_tiles or gq >= NQG:
                    return
                t = tp.tile([128, QG * W], F16)
                nc.scalar.activation(
                    out=t[:, :], in_=x4[:, gq * QG * W : (gq + 1) * QG * W],
                    func=mybir.ActivationFunctionType.Copy,
                    scale=cq_ap, bias=RND,
                )
                # offset removal also on scalar: PE's qx-ready wait must
                # not ride the vector queue behind PSUM evictions
                qt = qxp.tile([128, QG * W], BF16)
                with nc.allow_low_precision("int8 values exact in bf16"):
                    nc.scalar.activation(
                        out=qt[:, :], in_=t[:, :],
                        func=mybir.ActivationFunctionType.Copy, bias=-RND,
                    )
                qx_tiles[gq] = qt

            def neighbors(q):
                gq, off = divmod(q, QG)
                off *= W
                cur = qx_tiles[gq]
                if q == 0:
                    prev = None
                elif q % QG:
                    prev = (cur, off - W)
                else:
                    prev = (qx_tiles[gq - 1], (QG - 1) * W)
                if q == NQ - 1:
                    nxt = None
                elif (q % QG) != QG - 1:
                    nxt = (cur, off + W)
                else:
                    nxt = (qx_tiles[gq + 1], 0)
                return cur, off, prev, nxt

            quantize_group(0)
            quantize_group(1)

            for q0 in range(0, NQ, BI):
                qs = list(range(q0, min(q0 + BI, NQ)))
                # prefetch quantize ~2 interleave-groups ahead
                for qq in range(q0, min(q0 + 3 * BI + 1, NQ)):
                    quantize_group(qq // QG)

                psts = {}
                mains = {}
                bounds = {}
                for q in qs:
                    pst = psp.tile([128, W], F32, tag="pst")
                    psts[q] = pst
                    cur, off, prev, nxt = neighbors(q)
                    mains[q] = [
                        (w4[:, 128:256], cur[:, off : off + 512],
                         pst[:, 0:512], (0, 0)),
                        (w4[:, 0:128], cur[:, off : off + 511],
                         pst[:, 1:512], (0, 0)),
                        (w4[:, 256:384], cur[:, off + 1 : off + 512],
                         pst[:, 0:511], (0, 0)),
                    ]
                    bnd = []
                    if prev is not None:
                        pt, p0 = prev
                        bnd.append([
                            (bw[96:128, 32:64], pt[96:128, p0 : p0 + 512],
                             pst[0:32, 0:512], (96, 0)),
                            (bw[96:128, 0:32], pt[96:128, p0 : p0 + 511],
                             pst[0:32, 1:512], (96, 0)),
                            (bw[96:128, 64:96], pt[96:128, p0 + 1 : p0 + 512],
                             pst[0:32, 0:511], (96, 0)),
                        ])
                    if nxt is not None:
                        nt, n0 = nxt
                        bnd.append([
                            (bw[0:32, 32:64], nt[0:32, n0 : n0 + 512],
                             pst[96:128, 0:512], (0, 96)),
                            (bw[0:32, 0:32], nt[0:32, n0 : n0 + 511],
                             pst[96:128, 1:512], (0, 96)),
                            (bw[0:32, 64:96], nt[0:32, n0 + 1 : n0 + 512],
                             pst[96:128, 0:511], (0, 96)),
                        ])
                    # per-kw rounds: each round holds the sp/sn pair (or
                    # single at image edges) that runs concurrently on
                    # disjoint 32x32 subarrays
                    bounds[q] = list(zip(*bnd)) if bnd else []

                # emission order: same main lhsT across the BI blocks
                # back-to-back (one LDWEIGHTS per BI matmuls), then the
                # cheap-LDW boundary matmuls, kw-round-major with each
                # block's sp/sn pair adjacent.
                order = []
                for idx in range(3):
                    for q in qs:
                        order.append((q, mains[q][idx]))
                for idx in range(3):
                    for q in qs:
                        for mm in bounds[q][idx] if idx < len(bounds[q]) else ():
                            order.append((q, mm))
                counts = {q: 0 for q in qs}
                totals = {
                    q: len(mains[q]) + sum(len(r) for r in bounds[q])
                    for q in qs
                }
                for q, (lhsT, rhs, outap, tpos) in order:
                    counts[q] += 1
                    nc.tensor.matmul(
                        outap, lhsT, rhs,
                        start=(counts[q] == 1), stop=(counts[q] == totals[q]),
                        tile_position=tpos,
                    )

                for q in qs:
                    ot = outp.tile([128, W], F16)
                    nc.vector.tensor_scalar(
                        out=ot[:, :], in0=psts[q][:, :],
                        scalar1=dq_ap, scalar2=bias_sb[:, 0:1],
                        op0=mybir.AluOpType.mult, op1=mybir.AluOpType.add,
                    )
                    ov = out_ext[:, 4 * q : 4 * q + 4, :].rearrange(
                        "o hm w -> hm o w"
                    )
                    eng = (nc.gpsimd, nc.sync)[q % 2]
                    eng.dma_start(out=ov, in_=ot[:, :])

    nc.finalize()
    return nc


_NC_CACHE = {}


def make_in_maps(x, weight, bias):
    x = np.ascontiguousarray(x, dtype=np.float32)
    wt = np.ascontiguousarray(
        np.asarray(weight, dtype=np.float32).transpose(1, 2, 3, 0).reshape(C, -1)
    )
    bias4 = np.ascontiguousarray(
        np.tile(np.asarray(bias, dtype=np.float32), 4)
    )
    return [
        {"x": x[i], "wt": wt, "bias4": bias4} for i in range(N_CORES)
    ]


def kernel(x, weight, bias):
    if "nc" not in _NC_CACHE:
        _NC_CACHE["nc"] = build_nc()
    nc = _NC_CACHE["nc"]
    in_maps = make_in_maps(x, weight, bias)
    res = run_bass_kernel_spmd(nc, in_maps, core_ids=list(range(N_CORES)))
    outs = [
        np.asarray(res.results[i]["out"], dtype=np.float32)
        for i in range(N_CORES)
    ]
    return np.stack(outs, axis=0)


if __name__ == "__main__":
    build_nc(h=32)
    print("build ok")


# revision 27
# speedup vs baseline: 2.4417x; 1.0559x over previous
"""Quantized int8 conv2d (brevitas-style) on 8 TRN2 NeuronCores.

Data-parallel over batch (1 image / core). Per-tensor symmetric int8
quantization: local abs-max -> AllReduce(max) -> quantize -> 3x3 conv
(stride 1, pad 1) as PE col-tiled matmuls -> dequant + bias.

Design notes:
- Pass 1: first 16 row-blocks ride the two HWDGE rings (fp32 + scalar
  convert); the rest are SWDGE cast-DMAs (fp32->fp16 inline) straight
  into the fp16 SBUF cache, spread over all 16 SDMA engines. Vector
  abs-max in 4-block chunks pipelines behind the loads.
- The AllReduce(max) is split: AR1 covers the head of the image and its
  trigger is emitted after ALL load dma_starts, so the collective's
  entry barrier (inter-core start skew, observed 5-80us) overlaps the
  draining tail loads; AR2 over the tail then completes between
  already-synchronized cores in ~1us of wait.
- round(v) = fp16(v + 1536) - 1536 (fp16 RNE at the [1024,2048) binade
  has ulp=1 -> exact round-half-even). The subtract writes bf16 (ints
  <= 127 are exact) so matmuls run at the PE's full bf16 rate, and
  zero-padding at image borders needs no correction matmuls.
- Conv: output rows grouped by 4 (c = h%4), partition = 32*(h%4)+ch.
  Per block: 3 full K=128 matmuls (one per kw; h-taps folded into the
  (hm -> c) block structure of the lhsT) + up to 6 K=32 boundary
  matmuls that read the adjacent block's edge row DIRECTLY from its
  SBUF tile (partitions 96-127 / 0-31) via PE row/col tile placement
  (no staging DMA; sp/sn pairs run on disjoint 32x32 subarrays).
- Blocks are processed in interleaved groups of 3 (6 PSUM banks) so
  each main lhsT is loaded once per 3 matmuls: fewer LDWEIGHTS stalls
  keeps the PE activity dense enough for HAM to unthrottle to 2.4 GHz.
- fp16 output (halves store traffic; well within 2e-2), upcast on host.
  Stores alternate gpsimd/sync queues (never scalar, which quantizes).
- weight comes in host-pre-transposed as wt[i, (kh kw o)] so the SBUF
  replica loads are contiguous; bias comes host-replicated x4.
"""

import sys

if "/opt/trn_rl_repo" not in sys.path:
    sys.path.insert(0, "/opt/trn_rl_repo")

import numpy as np

import concourse.bass as bass
import concourse.bacc as bacc
import concourse.mybir as mybir
from concourse import tile
from concourse.bass_utils import run_bass_kernel_spmd

N_CORES = 8
C = 32
O = 32
H = 512
W = 512
F32 = mybir.dt.float32
F16 = mybir.dt.float16
BF16 = mybir.dt.bfloat16

MAXV = 127.0
RND = 1536.0


def build_nc(h=H):
    nc = bacc.Bacc(None, target_bir_lowering=False, debug=False)
    NQ = h // 4          # 4-row blocks
    LG = min(16, NQ)     # blocks per load group (16 -> 4MB cast loads)
    NLG = NQ // LG
    QG = min(4, NQ)      # blocks per quantize group
    NQG = NQ // QG
    BI = 4               # block interleave (= QG; 4-way boundary pairing)

    x_ext = nc.declare_dram_parameter("x", [C, h, W], F32, isOutput=False)
    # wt = weight.transpose(1,2,3,0).reshape(C,-1), prepared on host
    w_ext = nc.declare_dram_parameter("wt", [C, 3 * 3 * O], F32, isOutput=False)
    b_ext = nc.declare_dram_parameter("bias4", [128], F32, isOutput=False)
    out_ext = nc.declare_dram_parameter("out", [O, h, W], F16, isOutput=True)

    cc_in = nc.dram_tensor("cc_in", [128], F32)
    cc_out = nc.dram_tensor("cc_out", [128], F32, addr_space="Shared")
    cc_in2 = nc.dram_tensor("cc_in2", [128], F32)
    cc_out2 = nc.dram_tensor("cc_out2", [128], F32, addr_space="Shared")

    with tile.TileContext(nc) as tc:
        with (
            tc.tile_pool(name="persist", bufs=1) as persist,
            tc.tile_pool(name="tp", bufs=3) as tp,
            tc.tile_pool(name="qx", bufs=5) as qxp,
            tc.tile_pool(name="outp", bufs=8) as outp,
            tc.tile_pool(name="psp", bufs=8, space="PSUM") as psp,
        ):
            # ---------------- persistent SBUF tensors ----------------
            x4 = persist.tile([128, NQ * W], F16)
            maxes = persist.tile([128, NQG + 8], F32)
            wsb = persist.tile([128, 288], F32)
            tq = persist.tile([128, 288], F16)
            qw = persist.tile([128, 288], BF16)
            w4 = persist.tile([128, 3 * 128], BF16)  # main lhsT: kw blocks (c,o)
            w4r = persist.tile([128, 3 * 128], BF16)  # row-permuted (odd blocks)
            bw = persist.tile([128, 192], BF16)      # boundary lhsT (both placements)
            ones_l = persist.tile([1, 128], F32)
            bias_sb = persist.tile([128, 1], F32)
            gmax = persist.tile([128, 1], F32)
            gmax2 = persist.tile([128, 1], F32)
            gmaxr = persist.tile([1, 128], F32)
            gmaxr2 = persist.tile([1, 128], F32)
            sg1 = persist.tile([1, 1], F32)
            sg2 = persist.tile([1, 1], F32)
            wred = persist.tile([128, 1], F32)
            wredr = persist.tile([1, 128], F32)
            sg = persist.tile([1, 1], F32)
            sw_ = persist.tile([1, 1], F32)
            inv = persist.tile([1, 1], F32)
            invw = persist.tile([1, 1], F32)
            cwi = persist.tile([1, 1], F32)
            cqi = persist.tile([1, 1], F32)
            dqi = persist.tile([1, 1], F32)
            bc_in = persist.tile([1, 2], F32)
            bvec = persist.tile([128, 2], F32)
            cw_ap = persist.tile([128, 1], F32)

            # ---------------- weight path (local) --------------------
            for g in range(4):
                eng = (nc.sync, nc.scalar)[g % 2]
                eng.dma_start(out=wsb[32 * g : 32 * g + 32, :], in_=w_ext[:, :])
            nc.sync.dma_start(out=bias_sb[:, :], in_=b_ext[:, None])
            nc.vector.memset(ones_l[:, :], 1.0)
            nc.vector.memset(w4[:, :], 0.0)
            nc.vector.memset(w4r[:, :], 0.0)
            nc.vector.memset(bw[:, :], 0.0)

            # sw = max |w| (X-reduce, fold partitions to a row, reduce again)
            nc.vector.tensor_reduce(
                out=wred[:, :], in_=wsb[:, :], axis=mybir.AxisListType.X,
                op=mybir.AluOpType.max, apply_absolute_value=True,
            )
            nc.sync.dma_start(out=wredr[0:1, 0:128], in_=wred[:, 0:1])
            nc.vector.tensor_reduce(
                out=sw_[:, :], in_=wredr[:, :], axis=mybir.AxisListType.X,
                op=mybir.AluOpType.max,
            )
            nc.vector.reciprocal(invw[:, :], sw_[:, :])
            nc.vector.tensor_scalar_mul(cwi[:, :], invw[:, :], MAXV)

            bps = psp.tile([128, 512], F32, tag="pst")
            nc.tensor.matmul(bps[:, 0:1], ones_l[:, :], cwi[:, :])
            nc.vector.tensor_copy(cw_ap[:, :], bps[:, 0:1])

            # qw = round(w * 127/sw) via fp16 +1536 trick, then remove offset
            nc.scalar.activation(
                out=tq[:, :], in_=wsb[:, :],
                func=mybir.ActivationFunctionType.Copy,
                scale=cw_ap[:, 0:1], bias=RND,
            )
            with nc.allow_low_precision("int8 values exact in fp16/bf16"):
                nc.vector.tensor_scalar_add(qw[:, :], tq[:, :], -RND)
                # main lhsT: w4[32*hm+i, kw*128+c*32+o] = qw[o,i,hm-c+1,kw]
                for cix in range(4):
                    for kw in range(3):
                        for kh in range(3):
                            hm = cix + kh - 1
                            if not (0 <= hm <= 3):
                                continue
                            nc.vector.tensor_copy(
                                w4[32 * hm : 32 * hm + 32,
                                   kw * 128 + cix * 32 : kw * 128 + cix * 32 + 32],
                                qw[0:32, kh * 96 + kw * 32 : kh * 96 + kw * 32 + 32],
                            )
                            nc.vector.tensor_copy(
                                w4r[32 * (3 - hm) : 32 * (3 - hm) + 32,
                                    kw * 128 + cix * 32 : kw * 128 + cix * 32 + 32],
                                qw[0:32, kh * 96 + kw * 32 : kh * 96 + kw * 32 + 32],
                            )
                # boundary lhsT:
                #   bw[96+i, kw*32+o] = qw[o,i,kh=0,kw]  (sp: prev block row 3)
                #   bw[i,    kw*32+o] = qw[o,i,kh=2,kw]  (sn: next block row 0)
                for kw in range(3):
                    # cols 0-95: sp lhsT (kh=0) @ rows 96-127, sn (kh=2) @ 0-31
                    nc.vector.tensor_copy(
                        bw[96:128, kw * 32 : kw * 32 + 32],
                        qw[96:128, 0 * 96 + kw * 32 : 0 * 96 + kw * 32 + 32],
                    )
                    nc.vector.tensor_copy(
                        bw[0:32, kw * 32 : kw * 32 + 32],
                        qw[0:32, 2 * 96 + kw * 32 : 2 * 96 + kw * 32 + 32],
                    )
                    # cols 96-191: mirrored placements for even blocks whose
                    # reversed-neighbor rows live on the opposite strips
                    nc.vector.tensor_copy(
                        bw[0:32, 96 + kw * 32 : 96 + kw * 32 + 32],
                        qw[0:32, 0 * 96 + kw * 32 : 0 * 96 + kw * 32 + 32],
                    )
                    nc.vector.tensor_copy(
                        bw[96:128, 96 + kw * 32 : 96 + kw * 32 + 32],
                        qw[96:128, 2 * 96 + kw * 32 : 2 * 96 + kw * 32 + 32],
                    )

            # ------- pass 1: cast-DMA x to fp16 SBUF cache + absmax ---
            # 4 per-hm 3-dim DMAs per group (fp32->fp16 inline cast,
            # SWDGE); vector abs-max in <=QG-block chunks pipelines
            # behind. Groups taper at the end so the serial absmax tail
            # after the final load is short.
            if NQ > 32:
                gsizes = [16] * (NQ // 16 - 1) + [8, 4, 2, 2]
            else:
                gsizes = [LG] * NLG
            assert sum(gsizes) == NQ
            b0 = 0
            ci = 0
            for gs in gsizes:
                for hm in (0, 2, 1, 3):
                    xv = x_ext[
                        :, 4 * b0 + hm : 4 * (b0 + gs) : 4, :
                    ]
                    nc.gpsimd.dma_start(
                        out=x4[32 * hm : 32 * hm + 32,
                               b0 * W : (b0 + gs) * W].rearrange(
                            "p (j w) -> p j w", j=gs
                        ),
                        in_=xv.rearrange("i j w -> i j w"),
                    )
                for cb in range(b0, b0 + gs, QG):
                    ce = min(cb + QG, b0 + gs)
                    nc.vector.tensor_reduce(
                        out=maxes[:, ci : ci + 1],
                        in_=x4[:, cb * W : ce * W],
                        axis=mybir.AxisListType.X,
                        op=mybir.AluOpType.max, apply_absolute_value=True,
                    )
                    ci += 1
                b0 += gs
                # AR1: all-reduce the first ~75% of maxes while the tail
                # of the image still loads -- the collective's barrier
                # wait (inter-core skew) hides behind the DMA engines.
                if b0 == 96 and NQ > 32:
                    ci1 = ci
                    nc.vector.tensor_reduce(
                        out=gmax[:, :], in_=maxes[:, 0:ci1],
                        axis=mybir.AxisListType.X, op=mybir.AluOpType.max,
                    )
                    nc.sync.dma_start(out=cc_in[:, None], in_=gmax[:, :])
                    nc.gpsimd.collective_compute(
                        "AllReduce", mybir.AluOpType.max,
                        replica_groups=[list(range(N_CORES))],
                        ins=[cc_in[:].opt()], outs=[cc_out[:].opt()],
                    )

            # ------------- all-reduce(max), tail part ------------------
            if NQ > 32:
                nc.vector.tensor_reduce(
                    out=gmax2[:, :], in_=maxes[:, ci1:ci],
                    axis=mybir.AxisListType.X, op=mybir.AluOpType.max,
                )
                nc.sync.dma_start(out=cc_in2[:, None], in_=gmax2[:, :])
                # read AR1's result only now: anything waiting on AR1
                # earlier in an engine FIFO would stall the load tail
                nc.sync.dma_start(out=gmaxr[0:1, 0:128], in_=cc_out[None, :])
                nc.vector.tensor_reduce(
                    out=sg1[:, :], in_=gmaxr[:, :],
                    axis=mybir.AxisListType.X, op=mybir.AluOpType.max,
                )
                nc.gpsimd.collective_compute(
                    "AllReduce", mybir.AluOpType.max,
                    replica_groups=[list(range(N_CORES))],
                    ins=[cc_in2[:].opt()], outs=[cc_out2[:].opt()],
                )
                nc.sync.dma_start(out=gmaxr2[0:1, 0:128], in_=cc_out2[None, :])
                nc.vector.tensor_reduce(
                    out=sg2[:, :], in_=gmaxr2[:, :],
                    axis=mybir.AxisListType.X, op=mybir.AluOpType.max,
                )
                nc.vector.tensor_max(sg[:, :], sg1[:, :], sg2[:, :])
            else:
                nc.vector.tensor_reduce(
                    out=gmax[:, :], in_=maxes[:, 0:ci],
                    axis=mybir.AxisListType.X, op=mybir.AluOpType.max,
                )
                nc.sync.dma_start(out=cc_in[:, None], in_=gmax[:, :])
                nc.gpsimd.collective_compute(
                    "AllReduce", mybir.AluOpType.max,
                    replica_groups=[list(range(N_CORES))],
                    ins=[cc_in[:].opt()], outs=[cc_out[:].opt()],
                )
                nc.sync.dma_start(out=gmaxr[0:1, 0:128], in_=cc_out[None, :])
                nc.vector.tensor_reduce(
                    out=sg[:, :], in_=gmaxr[:, :], axis=mybir.AxisListType.X,
                    op=mybir.AluOpType.max,
                )
            nc.vector.reciprocal(inv[:, :], sg[:, :])
            nc.vector.tensor_scalar_mul(bc_in[:, 0:1], inv[:, :], MAXV)
            nc.vector.tensor_mul(dqi[:, :], sg[:, :], sw_[:, :])
            nc.vector.tensor_scalar_mul(
                bc_in[:, 1:2], dqi[:, :], 1.0 / (MAXV * MAXV)
            )
            bps2 = psp.tile([128, 512], F32, tag="pst")
            nc.tensor.matmul(bps2[:, 0:2], ones_l[:, :], bc_in[:, 0:2])
            nc.vector.tensor_copy(bvec[:, 0:2], bps2[:, 0:2])
            cq_ap = bvec[:, 0:1]
            dq_ap = bvec[:, 1:2]

            # ---------------- pass 2 ----------------------------------
            qx_tiles = {}

            def quantize_group(gq, parts=1, t_vec=False):
                if gq in qx_tiles or gq >= NQG:
                    return
                t = tp.tile([128, QG * W], F16)
                qt = qxp.tile([128, QG * W], BF16)
                step = QG * W // parts
                for p0 in range(0, QG * W, step):
                    src_ap = x4[:, gq * QG * W + p0 : gq * QG * W + p0 + step]
                    if t_vec:
                        # startup only (vector queue still empty): the
                        # fp16-rounding t-pass on vector pipelines with
                        # the scalar q-pass for lower first-MM latency
                        with nc.allow_low_precision("fp16 round trick"):
                            nc.vector.tensor_scalar(
                                out=t[:, p0 : p0 + step], in0=src_ap,
                                scalar1=cq_ap, scalar2=RND,
                                op0=mybir.AluOpType.mult,
                                op1=mybir.AluOpType.add,
                            )
                    else:
                        nc.scalar.activation(
                            out=t[:, p0 : p0 + step], in_=src_ap,
                            func=mybir.ActivationFunctionType.Copy,
                            scale=cq_ap, bias=RND,
                        )
                    # offset removal on scalar: PE's qx-ready wait must
                    # not ride the vector queue behind evictions
                    with nc.allow_low_precision("int8 values exact in bf16"):
                        nc.scalar.activation(
                            out=qt[:, p0 : p0 + step], in_=t[:, p0 : p0 + step],
                            func=mybir.ActivationFunctionType.Copy, bias=-RND,
                        )
                qx_tiles[gq] = qt

            def neighbors(q):
                gq, off = divmod(q, QG)
                off *= W
                cur = qx_tiles[gq]
                if q == 0:
                    prev = None
                elif q % QG:
                    prev = (cur, off - W)
                else:
                    prev = (qx_tiles[gq - 1], (QG - 1) * W)
                if q == NQ - 1:
                    nxt = None
                elif (q % QG) != QG - 1:
                    nxt = (cur, off + W)
                else:
                    nxt = (qx_tiles[gq + 1], 0)
                return cur, off, prev, nxt

            quantize_group(0, parts=4, t_vec=True)
            quantize_group(1, parts=2, t_vec=True)

            for q0 in range(0, NQ, BI):
                qs = list(range(q0, min(q0 + BI, NQ)))
                # prefetch quantize ~2 interleave-groups ahead
                for qq in range(q0, min(q0 + 2 * BI + 1, NQ)):
                    quantize_group(qq // QG)

                psts = {}
                mains = {}
                bounds = {}
                for q in qs:
                    pst = psp.tile([128, W], F32, tag="pst")
                    psts[q] = pst
                    cur, off, prev, nxt = neighbors(q)
                    wm = w4 if q % 2 == 0 else w4r
                    mains[q] = [
                        (wm[:, 128:256], cur[:, off : off + 512],
                         pst[:, 0:512], (0, 0)),
                        (wm[:, 0:128], cur[:, off : off + 511],
                         pst[:, 1:512], (0, 0)),
                        (wm[:, 256:384], cur[:, off + 1 : off + 512],
                         pst[:, 0:511], (0, 0)),
                    ]
                    bnd = []
                    if prev is not None:
                        pt, p0 = prev
                        if q % 2 == 1:
                            # prev even/normal: its row 3 lives at 96-127
                            bnd.append([
                                (bw[96:128, 32:64], pt[96:128, p0 : p0 + 512],
                                 pst[0:32, 0:512], (96, 0)),
                                (bw[96:128, 0:32], pt[96:128, p0 : p0 + 511],
                                 pst[0:32, 1:512], (96, 0)),
                                (bw[96:128, 64:96],
                                 pt[96:128, p0 + 1 : p0 + 512],
                                 pst[0:32, 0:511], (96, 0)),
                            ])
                        else:
                            # prev odd/reversed: its row 3 lives at 0-31
                            bnd.append([
                                (bw[0:32, 128:160], pt[0:32, p0 : p0 + 512],
                                 pst[0:32, 0:512], (0, 0)),
                                (bw[0:32, 96:128], pt[0:32, p0 : p0 + 511],
                                 pst[0:32, 1:512], (0, 0)),
                                (bw[0:32, 160:192],
                                 pt[0:32, p0 + 1 : p0 + 512],
                                 pst[0:32, 0:511], (0, 0)),
                            ])
                    if nxt is not None:
                        nt, n0 = nxt
                        if q % 2 == 1:
                            # next even/normal: its row 0 lives at 0-31
                            bnd.append([
                                (bw[0:32, 32:64], nt[0:32, n0 : n0 + 512],
                                 pst[96:128, 0:512], (0, 96)),
                                (bw[0:32, 0:32], nt[0:32, n0 : n0 + 511],
                                 pst[96:128, 1:512], (0, 96)),
                                (bw[0:32, 64:96], nt[0:32, n0 + 1 : n0 + 512],
                                 pst[96:128, 0:511], (0, 96)),
                            ])
                        else:
                            # next odd/reversed: its row 0 lives at 96-127
                            bnd.append([
                                (bw[96:128, 128:160],
                                 nt[96:128, n0 : n0 + 512],
                                 pst[96:128, 0:512], (96, 96)),
                                (bw[96:128, 96:128],
                                 nt[96:128, n0 : n0 + 511],
                                 pst[96:128, 1:512], (96, 96)),
                                (bw[96:128, 160:192],
                                 nt[96:128, n0 + 1 : n0 + 512],
                                 pst[96:128, 0:511], (96, 96)),
                            ])
                    # per-kw rounds: each round holds the sp/sn pair (or
                    # single at image edges) that runs concurrently on
                    # disjoint 32x32 subarrays
                    bounds[q] = list(zip(*bnd)) if bnd else []

                # emission order: same main lhsT across the BI blocks
                # back-to-back (one LDWEIGHTS per BI matmuls), then the
                # cheap-LDW boundary matmuls, kw-round-major with each
                # block's sp/sn pair adjacent.
                order = []
                for idx in range(3):
                    for q in qs:
                        order.append((q, mains[q][idx]))
                for idx in range(3):
                    for q in qs:
                        for mm in bounds[q][idx] if idx < len(bounds[q]) else ():
                            order.append((q, mm))
                counts = {q: 0 for q in qs}
                totals = {
                    q: len(mains[q]) + sum(len(r) for r in bounds[q])
                    for q in qs
                }
                for q, (lhsT, rhs, outap, tpos) in order:
                    counts[q] += 1
                    nc.tensor.matmul(
                        outap, lhsT, rhs,
                        start=(counts[q] == 1), stop=(counts[q] == totals[q]),
                        tile_position=tpos,
                    )

                for q in qs:
                    ot = outp.tile([128, W], F16)
                    if q >= NQ - 6 and q % 2 == 1:
                        # scalar is idle after the last quantize; split
                        # the final evictions across both engines
                        nc.scalar.activation(
                            out=ot[:, :], in_=psts[q][:, :],
                            func=mybir.ActivationFunctionType.Identity,
                            scale=dq_ap, bias=bias_sb[:, 0:1],
                        )
                    else:
                        nc.vector.tensor_scalar(
                            out=ot[:, :], in0=psts[q][:, :],
                            scalar1=dq_ap, scalar2=bias_sb[:, 0:1],
                            op0=mybir.AluOpType.mult, op1=mybir.AluOpType.add,
                        )
                    ov = out_ext[:, 4 * q : 4 * q + 4, :].rearrange(
                        "o hm w -> hm o w"
                    )
                    eng = (nc.gpsimd, nc.sync)[q % 2]
                    eng.dma_start(out=ov, in_=ot[:, :])

    nc.finalize()
    return nc


_NC_CACHE = {}


def make_in_maps(x, weight, bias):
    x = np.ascontiguousarray(x, dtype=np.float32)
    wt = np.ascontiguousarray(
        np.asarray(weight, dtype=np.float32).transpose(1, 2, 3, 0).reshape(C, -1)
    )
    bias4 = np.ascontiguousarray(
        np.tile(np.asarray(bias, dtype=np.float32), 4)
    )
    return [
        {"x": x[i], "wt": wt, "bias4": bias4} for i in range(N_CORES)
    ]


def kernel(x, weight, bias):
    if "nc" not in _NC_CACHE:
        _NC_CACHE["nc"] = build_nc()
    nc = _NC_CACHE["nc"]
    in_maps = make_in_maps(x, weight, bias)
    res = run_bass_kernel_spmd(nc, in_maps, core_ids=list(range(N_CORES)))
    outs = [
        np.asarray(res.results[i]["out"], dtype=np.float32)
        for i in range(N_CORES)
    ]
    return np.stack(outs, axis=0)


if __name__ == "__main__":
    build_nc(h=32)
    print("build ok")
